# revision 4
# baseline (speedup 1.0000x reference)
"""DeltaNet decode-step layer on 8 TRN2 NeuronCores (Bass/Tile).

Sharding (tensor-parallel over the 32 value heads + FFN intermediate):
  - core c owns value heads [4c, 4c+4) and key heads [2c, 2c+2)
  - in_proj rows / conv channels / z rows / a,b rows for those heads
  - out_proj columns for those heads (partial attn output, AllReduce'd)
  - FFN intermediate rows [1024c, 1024c+1024) (partial down output,
    summed on the host during unshard)

Weights are pre-transposed + packed on the host so every big matvec
streams the weight as the matmul *moving* operand in fp32r at
1 column/cycle:  pack[p, c, j] = W.T[c*128+p, j].

Device dataflow per core:
  rms1 -> in_proj matvec (fp32r) -> conv step -> delta rule (small fp32
  matmuls) -> gated rmsnorm -> out_proj matvec -> AllReduce(attn) ->
  rms2 -> gate/up matvec -> silu*mul -> down matvec -> partial ffn out.

A tiny dummy AllGather is issued first so the once-per-NEFF collective
entry barrier overlaps the in_proj weight streaming instead of the
real AllReduce.
"""
import numpy as np

import concourse.bacc as bacc
import concourse.mybir as mybir
import concourse.tile as tile
from concourse.bass_utils import run_bass_kernel_spmd

dt = mybir.dt
AF = mybir.ActivationFunctionType
ALU = mybir.AluOpType

H, I, QKV, TV = 2048, 8192, 8192, 4096
NVH, NKH, KD, VD, KC = 32, 16, 128, 128, 4
NC = 8
N_VH, N_KH = NVH // NC, NKH // NC          # 4 value heads, 2 key heads/core
CH = N_KH * KD * 2 + N_VH * VD             # 1024 conv channels/core
NZ = N_VH * VD                             # 512 z rows/core
R1 = CH + NZ + 2 * N_VH                    # 1544 in_proj rows/core
IC = I // NC                               # 1024 ffn rows/core
EPS = 1e-6
INV_SQRT_KD = float(KD) ** -0.5


def _pack_T(w_rows):
    """[J, H'] weight rows -> fp32 [128, H'/128, J] moving-operand pack."""
    t = np.ascontiguousarray(w_rows.T)                 # [H', J]
    hp, j = t.shape
    return np.ascontiguousarray(
        t.reshape(hp // 128, 128, j).transpose(1, 0, 2))


def _col16(v):
    """[H]-vector -> [128, H/128] column-major tile."""
    return np.ascontiguousarray(v.reshape(-1, 128).T)


def build_program():
    nc = bacc.Bacc("TRN2", target_bir_lowering=False, debug=False,
                   num_devices=NC)
    f32, f32r = dt.float32, dt.float32r

    # ---- per-core DRAM inputs ----
    w1 = nc.dram_tensor("w1", [128, 16, R1], f32r, kind="ExternalInput")
    wo = nc.dram_tensor("wo", [128, 4, H], f32r, kind="ExternalInput")
    wg = nc.dram_tensor("wg", [128, 16, IC], f32r, kind="ExternalInput")
    wu = nc.dram_tensor("wu", [128, 16, IC], f32r, kind="ExternalInput")
    wd = nc.dram_tensor("wd", [128, 8, H], f32r, kind="ExternalInput")
    xc = nc.dram_tensor("xc", [128, 16], f32, kind="ExternalInput")
    xr = nc.dram_tensor("xr", [1, H], f32, kind="ExternalInput")
    w1f = nc.dram_tensor("w1f", [128, 16], f32, kind="ExternalInput")
    w2f = nc.dram_tensor("w2f", [1, H], f32, kind="ExternalInput")
    convp = nc.dram_tensor("convp", [1, CH], f32, kind="ExternalInput")
    cwlast = nc.dram_tensor("cwlast", [1, CH], f32, kind="ExternalInput")
    ssm = nc.dram_tensor("ssm", [128, N_VH * VD], f32, kind="ExternalInput")
    scal = nc.dram_tensor("scal", [1, 8], f32, kind="ExternalInput")
    nw = nc.dram_tensor("nw", [1, NZ], f32, kind="ExternalInput")

    # ---- per-core DRAM outputs ----
    qkv_out = nc.dram_tensor("qkv_out", [1, CH], f32, kind="ExternalOutput")
    ssm_out = nc.dram_tensor("ssm_out", [128, N_VH * VD], f32,
                             kind="ExternalOutput")
    x1_out = nc.dram_tensor("x1_out", [1, H], f32, kind="ExternalOutput")
    ffn_out = nc.dram_tensor("ffn_out", [1, H], f32, kind="ExternalOutput")
    dbg_out = nc.dram_tensor("dbg_out", [NC, 4], f32, kind="ExternalOutput")

    with tile.TileContext(nc, trace_sim=False) as tc:
        with tc.tile_pool(name="wp", bufs=5) as wp, \
             tc.tile_pool(name="sp", bufs=1) as sp, \
             tc.tile_pool(name="pp", bufs=1, space="PSUM") as pp, \
             tc.tile_pool(name="dp", bufs=1, space="DRAM") as dp:

            # ---- dummy collective: absorb the once-per-NEFF barrier ----
            dz = sp.tile([1, 4], f32, name="dz")
            nc.vector.memset(dz[:], 1.0)
            d_in = dp.tile([1, 4], f32, name="d_in")
            d_out = dp.tile([NC, 4], f32, name="d_out")
            nc.scalar.dma_start(out=d_in[:], in_=dz[:])
            nc.gpsimd.collective_compute(
                "AllGather", ALU.bypass, ins=[d_in.opt()], outs=[d_out.opt()],
                replica_groups=[list(range(NC))])
            dzo = sp.tile([NC, 4], f32, name="dzo")
            nc.scalar.dma_start(out=dzo[:], in_=d_out[:])
            nc.scalar.dma_start(out=dbg_out[:], in_=dzo[:])

            # ---- small input loads (ACT HWDGE ring) ----
            xc_s = sp.tile([128, 16], f32, name="xc_s")
            nc.scalar.dma_start(out=xc_s[:], in_=xc[:])
            w1f_s = sp.tile([128, 16], f32, name="w1f_s")
            nc.scalar.dma_start(out=w1f_s[:], in_=w1f[:])
            scal_s = sp.tile([1, 8], f32, name="scal_s")
            nc.scalar.dma_start(out=scal_s[:], in_=scal[:])
            convp_s = sp.tile([1, CH], f32, name="convp_s")
            nc.scalar.dma_start(out=convp_s[:], in_=convp[:])
            cwlast_s = sp.tile([1, CH], f32, name="cwlast_s")
            nc.scalar.dma_start(out=cwlast_s[:], in_=cwlast[:])
            nw_s = sp.tile([1, NZ], f32, name="nw_s")
            nc.scalar.dma_start(out=nw_s[:], in_=nw[:])
            ssm_s = sp.tile([128, N_VH * VD], f32, name="ssm_s")
            nc.scalar.dma_start(out=ssm_s[:], in_=ssm[:])
            xr_s = sp.tile([1, H], f32, name="xr_s")
            nc.scalar.dma_start(out=xr_s[:], in_=xr[:])
            w2f_s = sp.tile([1, H], f32, name="w2f_s")
            nc.scalar.dma_start(out=w2f_s[:], in_=w2f[:])

            ones11 = sp.tile([1, 1], f32, name="ones11")
            nc.vector.memset(ones11[:], 1.0)
            ones_r = sp.tile([1, 128], f32, name="ones_r")
            nc.vector.memset(ones_r[:], 1.0)
            ones_c = sp.tile([128, 1], f32, name="ones_c")
            nc.vector.memset(ones_c[:], 1.0)
            eps1 = sp.tile([1, 1], f32, name="eps1")
            nc.vector.memset(eps1[:], EPS)

            # ---- stage A: h = rms(x) * (1 + rms1_w), column layout ----
            sqA = sp.tile([128, 16], f32, name="sqA")
            xacc = sp.tile([128, 1], f32, name="xacc")
            nc.scalar.activation(sqA[:], xc_s[:], AF.Square, accum_out=xacc[:])
            ps_s1 = pp.tile([1, 1], f32, tag="uno", bufs=4, name="ps_s1")
            nc.tensor.matmul(ps_s1[:], lhsT=ones_c[:], rhs=xacc[:],
                             start=True, stop=True)
            s1p = sp.tile([1, 1], f32, name="s1p")
            nc.scalar.activation(s1p[:], ps_s1[:], AF.Ln,
                                 scale=1.0 / H, bias=eps1[:])
            s1 = sp.tile([1, 1], f32, name="s1")
            nc.scalar.activation(s1[:], s1p[:], AF.Exp, scale=-0.5)
            ps_s1b = pp.tile([128, 1], f32, tag="uno", bufs=4, name="ps_s1b")
            nc.tensor.matmul(ps_s1b[:], lhsT=ones_r[:], rhs=s1[:],
                             start=True, stop=True)
            s1b = sp.tile([128, 1], f32, name="s1b")
            nc.scalar.copy(s1b[:], ps_s1b[:])
            h1 = sp.tile([128, 16], f32, name="h1")
            nc.vector.tensor_scalar(h1[:], xc_s[:], s1b[:], None, ALU.mult)
            h_r = sp.tile([128, 16], f32r, name="h_r")
            nc.vector.tensor_tensor(h_r[:], h1[:], w1f_s[:], ALU.mult)

            # ---- in_proj matvecs (fp32r, weight moving) ----
            # a/b rows first: decay/beta computed early off the critical path
            w1ab = wp.tile([128, 16, 2 * N_VH], f32r, tag="w", name="w1ab",
                           padded_shape=[128, 16, 256])
            nc.sync.dma_start(out=w1ab[:], in_=w1[:, :, CH + NZ:R1])
            ps_ab = pp.tile([1, 2 * N_VH], f32, tag="uno", bufs=4, name="ps_ab")
            for c in range(16):
                nc.tensor.matmul(ps_ab[:], lhsT=h_r[:, c:c + 1],
                                 rhs=w1ab[:, c, :],
                                 start=(c == 0), stop=(c == 15))
            ab_row = sp.tile([1, 2 * N_VH], f32, name="ab_row")
            nc.scalar.copy(ab_row[:], ps_ab[:])

            # qkv (4 j-tiles of 256) and z (2 j-tiles of 256)
            ps_qk = pp.tile([1, 512], f32, tag="uno", bufs=4, name="ps_qk")
            ps_v = pp.tile([1, 512], f32, tag="uno", bufs=4, name="ps_v")
            ps_z = pp.tile([1, 512], f32, tag="uno", bufs=4, name="ps_z")
            ps_of = {0: (ps_qk, 0), 1: (ps_qk, 256), 2: (ps_v, 0),
                     3: (ps_v, 256), 4: (ps_z, 0), 5: (ps_z, 256)}
            for jt in range(6):
                wt = wp.tile([128, 16, 256], f32r, tag="w", name="w1t")
                nc.sync.dma_start(out=wt[:], in_=w1[:, :, jt * 256:(jt + 1) * 256])
                ps, off = ps_of[jt]
                for c in range(16):
                    nc.tensor.matmul(ps[0:1, off:off + 256],
                                     lhsT=h_r[:, c:c + 1], rhs=wt[:, c, :],
                                     start=(c == 0), stop=(c == 15))

            qkv_row = sp.tile([1, CH], f32, name="qkv_row")
            nc.scalar.copy(qkv_row[0:1, 0:512], ps_qk[:])
            nc.scalar.copy(qkv_row[0:1, 512:1024], ps_v[:])
            nc.scalar.dma_start(out=qkv_out[:], in_=qkv_row[:])
            z_row = sp.tile([1, NZ], f32, name="z_row")
            nc.vector.tensor_copy(z_row[:], ps_z[:])

            # ---- decay / beta (from a,b rows) ----
            beta = sp.tile([1, N_VH], f32, name="beta")
            nc.scalar.activation(beta[:], ab_row[0:1, N_VH:2 * N_VH], AF.Exp,
                                 scale=-1.0)
            nc.vector.tensor_scalar_add(beta[:], beta[:], 1.0)
            nc.vector.reciprocal(beta[:], beta[:])
            spin = sp.tile([1, N_VH], f32, name="spin")
            nc.vector.tensor_tensor(spin[:], ab_row[0:1, 0:N_VH],
                                    scal_s[0:1, 4:8], ALU.add)
            sp_t = sp.tile([1, N_VH], f32, name="sp_t")
            nc.scalar.activation(sp_t[:], spin[:], AF.Exp)
            nc.vector.tensor_scalar_add(sp_t[:], sp_t[:], 1.0)
            nc.scalar.activation(sp_t[:], sp_t[:], AF.Ln)
            eA = sp.tile([1, N_VH], f32, name="eA")
            nc.scalar.activation(eA[:], scal_s[0:1, 0:4], AF.Exp)
            gabs = sp.tile([1, N_VH], f32, name="gabs")
            nc.vector.tensor_tensor(gabs[:], eA[:], sp_t[:], ALU.mult)
            decay = sp.tile([1, N_VH], f32, name="decay")
            nc.scalar.activation(decay[:], gabs[:], AF.Exp, scale=-1.0)
            ps_db = pp.tile([128, N_VH], f32, tag="uno", bufs=4, name="ps_db")
            nc.tensor.matmul(ps_db[:], lhsT=ones_r[:], rhs=decay[:],
                             start=True, stop=True)
            decay_bc = sp.tile([128, N_VH], f32, name="decay_bc")
            nc.scalar.copy(decay_bc[:], ps_db[:])
            decayed = sp.tile([128, N_VH * VD], f32, name="decayed")
            for vh in range(N_VH):
                nc.vector.tensor_scalar(
                    decayed[:, vh * VD:(vh + 1) * VD],
                    ssm_s[:, vh * VD:(vh + 1) * VD],
                    decay_bc[:, vh:vh + 1], None, ALU.mult)

            # ---- conv step + silu ----
            convsum = sp.tile([1, CH], f32, name="convsum")
            nc.vector.tensor_tensor(convsum[:], qkv_row[:], cwlast_s[:], ALU.mult)
            nc.vector.tensor_tensor(convsum[:], convsum[:], convp_s[:], ALU.add)
            conv_row = sp.tile([1, CH], f32, name="conv_row")
            nc.scalar.activation(conv_row[:], convsum[:], AF.Exp, scale=-1.0)
            nc.vector.tensor_scalar_add(conv_row[:], conv_row[:], 1.0)
            nc.vector.reciprocal(conv_row[:], conv_row[:])
            nc.vector.tensor_tensor(conv_row[:], conv_row[:], convsum[:],
                                    ALU.mult)

            # ---- l2-normalize q, k per key head ----
            qkn_row = sp.tile([1, 512], f32, name="qkn_row")
            scr = sp.tile([1, H], f32, name="scr")     # shared scratch row
            for i, (off, qscale) in enumerate(
                    [(0, True), (128, True), (256, False), (384, False)]):
                vec = conv_row[0:1, off:off + 128]
                ss = sp.tile([1, 1], f32, tag="ss_t", name="ss")
                nc.scalar.activation(scr[0:1, 0:128], vec, AF.Square,
                                     accum_out=ss[:])
                nc.vector.tensor_scalar_max(ss[:], ss[:], 1e-24)
                ns = sp.tile([1, 1], f32, tag="ns_t", name="ns")
                nc.scalar.activation(ns[:], ss[:], AF.Ln)
                inv = sp.tile([1, 1], f32, tag="inv_t", name="inv")
                nc.scalar.activation(inv[:], ns[:], AF.Exp, scale=-0.5)
                if qscale:
                    nc.vector.tensor_scalar(qkn_row[0:1, off:off + 128], vec,
                                            inv[:], INV_SQRT_KD,
                                            ALU.mult, ALU.mult)
                else:
                    nc.vector.tensor_scalar(qkn_row[0:1, off:off + 128], vec,
                                            inv[:], None, ALU.mult)

            # ---- transpose q0,q1,k0,k1 rows -> columns ----
            ps_qkT = pp.tile([128, 4], f32, tag="uno", bufs=4, name="ps_qkT")
            for i in range(4):
                nc.tensor.transpose(ps_qkT[:, i:i + 1],
                                    qkn_row[0:1, i * 128:(i + 1) * 128],
                                    ones11[:])
            qk_cols = sp.tile([128, 4], f32, name="qk_cols")
            nc.scalar.copy(qk_cols[:], ps_qkT[:])

            # ---- delta rule ----
            ps_sk = pp.tile([1, 512], f32, tag="uno", bufs=4, name="ps_sk")
            for p in range(2):          # key-head pair p covers vh 2p, 2p+1
                nc.tensor.matmul(ps_sk[0:1, p * 256:(p + 1) * 256],
                                 lhsT=qk_cols[:, 2 + p:3 + p],
                                 rhs=decayed[:, p * 256:(p + 1) * 256],
                                 start=True, stop=True)
            delta_row = sp.tile([1, 512], f32, name="delta_row")
            nc.vector.tensor_tensor(delta_row[:], conv_row[0:1, 512:1024],
                                    ps_sk[:], ALU.subtract)
            for vh in range(N_VH):
                nc.vector.tensor_scalar(delta_row[0:1, vh * 128:(vh + 1) * 128],
                                        delta_row[0:1, vh * 128:(vh + 1) * 128],
                                        beta[0:1, vh:vh + 1], None, ALU.mult)
            ps_ssm = pp.tile([128, 512], f32, tag="uno", bufs=4, name="ps_ssm")
            for vh in range(N_VH):
                nc.tensor.matmul(ps_ssm[:, vh * 128:(vh + 1) * 128],
                                 lhsT=qkn_row[0:1, (256 + (vh // 2) * 128):
                                              (384 + (vh // 2) * 128)],
                                 rhs=delta_row[0:1, vh * 128:(vh + 1) * 128],
                                 start=True, stop=True)
            new_ssm = sp.tile([128, 512], f32, name="new_ssm")
            nc.vector.tensor_tensor(new_ssm[:], ps_ssm[:], decayed[:], ALU.add)
            nc.scalar.dma_start(out=ssm_out[:], in_=new_ssm[:])
            ps_y = pp.tile([1, 512], f32, tag="uno", bufs=4, name="ps_y")
            for p in range(2):
                nc.tensor.matmul(ps_y[0:1, p * 256:(p + 1) * 256],
                                 lhsT=qk_cols[:, p:p + 1],
                                 rhs=new_ssm[:, p * 256:(p + 1) * 256],
                                 start=True, stop=True)
            y_row = sp.tile([1, 512], f32, name="y_row")
            nc.vector.tensor_copy(y_row[:], ps_y[:])

            # ---- gated rmsnorm + silu(z) gate ----
            for vh in range(N_VH):
                yv = y_row[0:1, vh * 128:(vh + 1) * 128]
                ys = sp.tile([1, 1], f32, tag="ys_t", name="ys")
                nc.scalar.activation(scr[0:1, 0:128], yv, AF.Square,
                                     accum_out=ys[:])
                yn = sp.tile([1, 1], f32, tag="yn_t", name="yn")
                nc.scalar.activation(yn[:], ys[:], AF.Ln,
                                     scale=1.0 / VD, bias=eps1[:])
                yi = sp.tile([1, 1], f32, tag="yi_t", name="yi")
                nc.scalar.activation(yi[:], yn[:], AF.Exp, scale=-0.5)
                nc.vector.tensor_scalar(yv, yv, yi[:], None, ALU.mult)
            nc.vector.tensor_tensor(y_row[:], y_row[:], nw_s[:], ALU.mult)
            zs_row = sp.tile([1, 512], f32, name="zs_row")
            nc.scalar.activation(zs_row[:], z_row[:], AF.Exp, scale=-1.0)
            nc.vector.tensor_scalar_add(zs_row[:], zs_row[:], 1.0)
            nc.vector.reciprocal(zs_row[:], zs_row[:])
            nc.vector.tensor_tensor(zs_row[:], zs_row[:], z_row[:], ALU.mult)
            yout_row = sp.tile([1, 512], f32, name="yout_row")
            nc.vector.tensor_tensor(yout_row[:], y_row[:], zs_row[:], ALU.mult)
            ps_yo = pp.tile([128, 4], f32, tag="uno", bufs=4, name="ps_yo")
            for i in range(4):
                nc.tensor.transpose(ps_yo[:, i:i + 1],
                                    yout_row[0:1, i * 128:(i + 1) * 128],
                                    ones11[:])
            yo_col = sp.tile([128, 4], f32r, name="yo_col")
            nc.scalar.copy(yo_col[:], ps_yo[:])

            # ---- out_proj partial: [1,2048] ----
            ps_attn = pp.tile([1, H], f32, tag="big4", bufs=1, name="ps_attn")
            for half in range(2):
                wot = wp.tile([128, 4, 1024], f32r, tag="w", name="wot")
                nc.sync.dma_start(out=wot[:],
                                  in_=wo[:, :, half * 1024:(half + 1) * 1024])
                for sub in range(2):
                    off = half * 1024 + sub * 512
                    for c in range(4):
                        nc.tensor.matmul(
                            ps_attn[0:1, off:off + 512],
                            lhsT=yo_col[:, c:c + 1],
                            rhs=wot[:, c, sub * 512:(sub + 1) * 512],
                            start=(c == 0), stop=(c == 3))
            attn_row = sp.tile([1, H], f32, name="attn_row")
            nc.scalar.copy(attn_row[0:1, 0:1024], ps_attn[0:1, 0:1024])
            nc.scalar.copy(attn_row[0:1, 1024:2048], ps_attn[0:1, 1024:2048])

            # ---- AllReduce attn partials ----
            ar_in = dp.tile([1, H], f32, name="ar_in")
            ar_out = dp.tile([1, H], f32, name="ar_out")
            nc.scalar.dma_start(out=ar_in[:], in_=attn_row[:])
            nc.gpsimd.collective_compute(
                "AllReduce", ALU.add, ins=[ar_in.opt()], outs=[ar_out.opt()],
                replica_groups=[list(range(NC))])
            ar_row = sp.tile([1, H], f32, name="ar_row")
            nc.scalar.dma_start(out=ar_row[:], in_=ar_out[:])

            # ---- residual + rms2 ----
            nc.vector.tensor_tensor(ar_row[:], ar_row[:], xr_s[:], ALU.add)
            nc.scalar.dma_start(out=x1_out[:], in_=ar_row[:])
            s2a = sp.tile([1, 1], f32, name="s2a")
            nc.scalar.activation(scr[:], ar_row[:], AF.Square, accum_out=s2a[:])
            s2p = sp.tile([1, 1], f32, name="s2p")
            nc.scalar.activation(s2p[:], s2a[:], AF.Ln, scale=1.0 / H,
                                 bias=eps1[:])
            s2 = sp.tile([1, 1], f32, name="s2")
            nc.scalar.activation(s2[:], s2p[:], AF.Exp, scale=-0.5)
            h2_row = sp.tile([1, H], f32, name="h2_row")
            nc.vector.tensor_scalar(h2_row[:], ar_row[:], s2[:], None, ALU.mult)
            nc.vector.tensor_tensor(h2_row[:], h2_row[:], w2f_s[:], ALU.mult)
            ps_h2 = pp.tile([128, 16], f32, tag="uno", bufs=4, name="ps_h2")
            for i in range(16):
                nc.tensor.transpose(ps_h2[:, i:i + 1],
                                    h2_row[0:1, i * 128:(i + 1) * 128],
                                    ones11[:])
            h2c = sp.tile([128, 16], f32r, name="h2c")
            nc.scalar.copy(h2c[:], ps_h2[:])

            # ---- FFN gate/up matvecs ----
            ps_gu = pp.tile([1, 2 * IC], f32, tag="big4", bufs=1, name="ps_gu")
            for jt in range(4):
                wgt = wp.tile([128, 16, 256], f32r, tag="w", name="wgt")
                nc.sync.dma_start(out=wgt[:], in_=wg[:, :, jt * 256:(jt + 1) * 256])
                for c in range(16):
                    nc.tensor.matmul(ps_gu[0:1, jt * 256:(jt + 1) * 256],
                                     lhsT=h2c[:, c:c + 1], rhs=wgt[:, c, :],
                                     start=(c == 0), stop=(c == 15))
                wut = wp.tile([128, 16, 256], f32r, tag="w", name="wut")
                nc.sync.dma_start(out=wut[:], in_=wu[:, :, jt * 256:(jt + 1) * 256])
                for c in range(16):
                    nc.tensor.matmul(ps_gu[0:1, IC + jt * 256:IC + (jt + 1) * 256],
                                     lhsT=h2c[:, c:c + 1], rhs=wut[:, c, :],
                                     start=(c == 0), stop=(c == 15))
            s_row = sp.tile([1, IC], f32, name="s_row")
            gs_row = sp.tile([1, 256], f32, name="gs_row")
            for jt in range(4):
                gsl = s_row[0:1, jt * 256:(jt + 1) * 256]
                nc.scalar.activation(gs_row[:], ps_gu[0:1, jt * 256:(jt + 1) * 256],
                                     AF.Exp, scale=-1.0)
                nc.vector.tensor_scalar_add(gs_row[:], gs_row[:], 1.0)
                nc.vector.reciprocal(gs_row[:], gs_row[:])
                nc.vector.tensor_tensor(gs_row[:], gs_row[:],
                                        ps_gu[0:1, jt * 256:(jt + 1) * 256],
                                        ALU.mult)
                nc.vector.tensor_tensor(gsl, gs_row[:],
                                        ps_gu[0:1, IC + jt * 256:IC + (jt + 1) * 256],
                                        ALU.mult)
            ps_sT = pp.tile([128, 8], f32, tag="uno", bufs=4, name="ps_sT")
            for i in range(8):
                nc.tensor.transpose(ps_sT[:, i:i + 1],
                                    s_row[0:1, i * 128:(i + 1) * 128],
                                    ones11[:])
            s_col = sp.tile([128, 8], f32r, name="s_col")
            nc.scalar.copy(s_col[:], ps_sT[:])

            # ---- FFN down matvec -> partial [1,2048] ----
            ps_ffn = pp.tile([1, H], f32, tag="big4", bufs=1, name="ps_ffn")
            for jt in range(4):
                wdt = wp.tile([128, 8, 512], f32r, tag="w", name="wdt")
                nc.sync.dma_start(out=wdt[:], in_=wd[:, :, jt * 512:(jt + 1) * 512])
                for c in range(8):
                    nc.tensor.matmul(ps_ffn[0:1, jt * 512:(jt + 1) * 512],
                                     lhsT=s_col[:, c:c + 1], rhs=wdt[:, c, :],
                                     start=(c == 0), stop=(c == 7))
            ffn_row = sp.tile([1, H], f32, name="ffn_row")
            nc.scalar.copy(ffn_row[0:1, 0:1024], ps_ffn[0:1, 0:1024])
            nc.scalar.copy(ffn_row[0:1, 1024:2048], ps_ffn[0:1, 1024:2048])
            nc.scalar.dma_start(out=ffn_out[:], in_=ffn_row[:])

    nc.compile()
    return nc


_CACHE = {}


def _get_program():
    if "nc" not in _CACHE:
        _CACHE["nc"] = build_program()
    return _CACHE["nc"]


def _shard_inputs(x, conv_state, ssm_state, in_proj_w, out_proj_w, conv_weight,
                  A_log, dt_bias, norm_weight, rms1_w, rms2_w, gate_w, up_w,
                  down_w):
    """Pack + shard full inputs into per-core input maps."""
    f = np.float32
    in_maps = []
    ch_idx_all = []
    for c in range(NC):
        q_rows = np.arange(256 * c, 256 * (c + 1))
        k_rows = 2048 + q_rows
        v_rows = np.arange(4096 + 512 * c, 4096 + 512 * (c + 1))
        z_rows = np.arange(QKV + 512 * c, QKV + 512 * (c + 1))
        a_rows = np.arange(QKV + TV + 4 * c, QKV + TV + 4 * (c + 1))
        b_rows = a_rows + NVH
        rows1 = np.concatenate([q_rows, k_rows, v_rows, z_rows, a_rows, b_rows])
        ch_idx = np.concatenate([q_rows, k_rows, v_rows])  # conv channels
        ch_idx_all.append(ch_idx)
        vs = slice(512 * c, 512 * (c + 1))
        ics = slice(IC * c, IC * (c + 1))
        vh = slice(4 * c, 4 * (c + 1))

        cs = conv_state[ch_idx]          # [1024, 4]
        cw = conv_weight[ch_idx]         # [1024, 4]
        in_maps.append({
            "w1": _pack_T(in_proj_w[rows1]),
            "wo": _pack_T(out_proj_w[:, vs]),
            "wg": _pack_T(gate_w[ics]),
            "wu": _pack_T(up_w[ics]),
            "wd": _pack_T(down_w[:, ics]),
            "xc": _col16(x[0]).astype(f),
            "xr": x.astype(f),
            "w1f": _col16(1.0 + rms1_w).astype(f),
            "w2f": (1.0 + rms2_w)[None, :].astype(f),
            "convp": np.sum(cs[:, 1:4] * cw[:, 0:3], axis=1)[None, :].astype(f),
            "cwlast": cw[:, 3][None, :].astype(f),
            "ssm": np.concatenate([ssm_state[i] for i in range(4 * c, 4 * c + 4)],
                                  axis=1).astype(f),
            "scal": np.concatenate([A_log[vh], dt_bias[vh]])[None, :].astype(f),
            "nw": np.tile(norm_weight, N_VH)[None, :].astype(f),
        })
    return in_maps, ch_idx_all


def kernel(**inputs):
    inputs = {k: np.asarray(v) for k, v in inputs.items()}
    nc = _get_program()
    in_maps, ch_idx_all = _shard_inputs(**inputs)
    res = run_bass_kernel_spmd(nc, in_maps, core_ids=list(range(NC))).results

    # ---- unshard ----
    x_final = res[0]["x1_out"].astype(np.float64)
    for c in range(NC):
        x_final = x_final + res[c]["ffn_out"].astype(np.float64)
    x_final = x_final.astype(np.float32)

    conv_state = inputs["conv_state"]
    new_conv = np.empty((QKV, KC), np.float32)
    new_conv[:, 0:3] = conv_state[:, 1:4]
    for c in range(NC):
        new_conv[ch_idx_all[c], 3] = res[c]["qkv_out"][0]

    new_ssm = np.empty((NVH, KD, VD), np.float32)
    for c in range(NC):
        blk = res[c]["ssm_out"].reshape(KD, N_VH, VD).transpose(1, 0, 2)
        new_ssm[4 * c:4 * (c + 1)] = blk

    return x_final, new_conv, new_ssm


# revision 7
# speedup vs baseline: 1.2452x; 1.2452x over previous
"""DeltaNet decode-step layer on 8 TRN2 NeuronCores (Bass/Tile).

Sharding (tensor-parallel over the 32 value heads + FFN intermediate):
  - core c owns value heads [4c, 4c+4) and key heads [2c, 2c+2)
  - in_proj rows / conv channels / z rows / a,b rows for those heads
  - out_proj columns for those heads (partial attn output, AllReduce'd)
  - FFN intermediate rows [1024c, 1024c+1024) (partial down output,
    summed on the host during unshard)

Weights are pre-transposed + packed on the host so every big matvec
streams the weight as the matmul *moving* operand in fp32r at
1 column/cycle:  pack[p, c, j] = W.T[c*128+p, j].

Elementwise chains run in partition-parallel "column" layouts
([128, nheads] tiles, one head per free column) — single-partition
[1, N] ops process only one lane/cycle and would dominate the
critical path otherwise.  PE transposes move rows<->columns.

A tiny dummy AllGather is issued first so the once-per-NEFF collective
entry barrier overlaps the in_proj weight streaming instead of the
real AllReduce.
"""
import numpy as np

import concourse.bacc as bacc
import concourse.mybir as mybir
import concourse.tile as tile
from concourse.bass_utils import run_bass_kernel_spmd

dt = mybir.dt
AF = mybir.ActivationFunctionType
ALU = mybir.AluOpType

H, I, QKV, TV = 2048, 8192, 8192, 4096
NVH, NKH, KD, VD, KC = 32, 16, 128, 128, 4
NC = 8
N_VH, N_KH = NVH // NC, NKH // NC          # 4 value heads, 2 key heads/core
CH = N_KH * KD * 2 + N_VH * VD             # 1024 conv channels/core
NZ = N_VH * VD                             # 512 z rows/core
R1 = CH + NZ + 2 * N_VH                    # 1544 in_proj rows/core
IC = I // NC                               # 1024 ffn rows/core
EPS = 1e-6
INV_SQRT_KD = float(KD) ** -0.5


def _pack_T(w_rows):
    """[J, H'] weight rows -> fp32 [128, H'/128, J] moving-operand pack."""
    t = np.ascontiguousarray(w_rows.T)                 # [H', J]
    hp, j = t.shape
    return np.ascontiguousarray(
        t.reshape(hp // 128, 128, j).transpose(1, 0, 2))


def _colmaj(v):
    """[N]-vector -> [128, N/128] column-major tile (elem i -> [i%128, i//128])."""
    return np.ascontiguousarray(v.reshape(-1, 128).T)


def build_program():
    nc = bacc.Bacc("TRN2", target_bir_lowering=False, debug=False,
                   num_devices=NC)
    f32, f32r = dt.float32, dt.float32r

    # ---- per-core DRAM inputs ----
    w1 = nc.dram_tensor("w1", [128, 16, R1], f32r, kind="ExternalInput")
    wo = nc.dram_tensor("wo", [128, 4, H], f32r, kind="ExternalInput")
    wg = nc.dram_tensor("wg", [128, 16, IC], f32r, kind="ExternalInput")
    wu = nc.dram_tensor("wu", [128, 16, IC], f32r, kind="ExternalInput")
    wd = nc.dram_tensor("wd", [128, 8, H], f32r, kind="ExternalInput")
    xc = nc.dram_tensor("xc", [128, 16], f32, kind="ExternalInput")
    w1f = nc.dram_tensor("w1f", [128, 16], f32, kind="ExternalInput")
    w2f = nc.dram_tensor("w2f", [128, 16], f32, kind="ExternalInput")
    convp = nc.dram_tensor("convp", [128, 8], f32, kind="ExternalInput")
    cwlast = nc.dram_tensor("cwlast", [128, 8], f32, kind="ExternalInput")
    ssm = nc.dram_tensor("ssm", [128, N_VH * VD], f32, kind="ExternalInput")
    scal = nc.dram_tensor("scal", [1, 8], f32, kind="ExternalInput")
    qmask = nc.dram_tensor("qmask", [1, 4], f32, kind="ExternalInput")
    nwc = nc.dram_tensor("nwc", [128, 1], f32, kind="ExternalInput")
    ident = nc.dram_tensor("ident", [128, 128], f32, kind="ExternalInput")

    # ---- per-core DRAM outputs ----
    qkv_out = nc.dram_tensor("qkv_out", [128, 8], f32, kind="ExternalOutput")
    ssm_out = nc.dram_tensor("ssm_out", [128, N_VH * VD], f32,
                             kind="ExternalOutput")
    x1_out = nc.dram_tensor("x1_out", [128, 16], f32, kind="ExternalOutput")
    ffn_out = nc.dram_tensor("ffn_out", [1, H], f32, kind="ExternalOutput")
    dbg_out = nc.dram_tensor("dbg_out", [NC, 4], f32, kind="ExternalOutput")

    with tile.TileContext(nc, trace_sim=False) as tc:
        with tc.tile_pool(name="wp", bufs=4) as wp, \
             tc.tile_pool(name="sp", bufs=1) as sp, \
             tc.tile_pool(name="pp", bufs=1, space="PSUM") as pp, \
             tc.tile_pool(name="dp", bufs=1, space="DRAM") as dp:

            def puno(shape, name):
                return pp.tile(shape, f32, tag="uno", bufs=4, name=name)

            # ---- dummy collective: absorb the once-per-NEFF barrier ----
            dz = sp.tile([1, 4], f32, name="dz")
            nc.vector.memset(dz[:], 1.0)
            d_in = dp.tile([1, 4], f32, name="d_in")
            d_out = dp.tile([NC, 4], f32, name="d_out")
            nc.scalar.dma_start(out=d_in[:], in_=dz[:])
            nc.gpsimd.collective_compute(
                "AllGather", ALU.bypass, ins=[d_in.opt()], outs=[d_out.opt()],
                replica_groups=[list(range(NC))])
            dzo = sp.tile([NC, 4], f32, name="dzo")
            nc.scalar.dma_start(out=dzo[:], in_=d_out[:])
            nc.scalar.dma_start(out=dbg_out[:], in_=dzo[:])

            # ---- small input loads (ACT HWDGE ring) ----
            def load(name, shape, src):
                t = sp.tile(shape, f32, name=name)
                nc.scalar.dma_start(out=t[:], in_=src[:])
                return t
            xc_s = load("xc_s", [128, 16], xc)
            w1f_s = load("w1f_s", [128, 16], w1f)
            scal_s = load("scal_s", [1, 8], scal)
            qmask_s = load("qmask_s", [1, 4], qmask)
            convp_s = load("convp_s", [128, 8], convp)
            cwlast_s = load("cwlast_s", [128, 8], cwlast)
            nw_s = load("nw_s", [128, 1], nwc)
            ssm_s = load("ssm_s", [128, N_VH * VD], ssm)
            w2f_s = load("w2f_s", [128, 16], w2f)
            id_s = load("id_s", [128, 128], ident)

            ones11 = sp.tile([1, 1], f32, name="ones11")
            nc.vector.memset(ones11[:], 1.0)
            ones_r = sp.tile([1, 128], f32, name="ones_r")
            nc.vector.memset(ones_r[:], 1.0)
            ones_c = sp.tile([128, 1], f32, name="ones_c")
            nc.vector.memset(ones_c[:], 1.0)
            eps1 = sp.tile([1, 1], f32, name="eps1")
            nc.vector.memset(eps1[:], EPS)

            # ---- stage A: h = rms(x) * (1 + rms1_w), column layout ----
            sqA = sp.tile([128, 16], f32, name="sqA")
            xacc = sp.tile([128, 1], f32, name="xacc")
            nc.scalar.activation(sqA[:], xc_s[:], AF.Square, accum_out=xacc[:])
            ps_s1 = puno([1, 1], "ps_s1")
            nc.tensor.matmul(ps_s1[:], lhsT=ones_c[:], rhs=xacc[:],
                             start=True, stop=True)
            s1p = sp.tile([1, 1], f32, name="s1p")
            nc.scalar.activation(s1p[:], ps_s1[:], AF.Ln,
                                 scale=1.0 / H, bias=eps1[:])
            s1 = sp.tile([1, 1], f32, name="s1")
            nc.scalar.activation(s1[:], s1p[:], AF.Exp, scale=-0.5)
            ps_s1b = puno([128, 1], "ps_s1b")
            nc.tensor.matmul(ps_s1b[:], lhsT=ones_r[:], rhs=s1[:],
                             start=True, stop=True)
            s1b = sp.tile([128, 1], f32, name="s1b")
            nc.scalar.copy(s1b[:], ps_s1b[:])
            h1 = sp.tile([128, 16], f32, name="h1")
            nc.vector.tensor_scalar(h1[:], xc_s[:], s1b[:], None, ALU.mult)
            h_r = sp.tile([128, 16], f32r, name="h_r")
            nc.vector.tensor_tensor(h_r[:], h1[:], w1f_s[:], ALU.mult)

            # ---- in_proj matvecs (fp32r, weight moving) ----
            # a/b rows first: decay/beta computed early off the critical path
            w1ab = wp.tile([128, 16, 2 * N_VH], f32r, tag="w", name="w1ab")
            nc.sync.dma_start(out=w1ab[:], in_=w1[:, :, CH + NZ:R1])
            ps_ab = puno([1, 2 * N_VH], "ps_ab")
            for c in range(16):
                nc.tensor.matmul(ps_ab[:], lhsT=h_r[:, c:c + 1],
                                 rhs=w1ab[:, c, :],
                                 start=(c == 0), stop=(c == 15))
            ab_row = sp.tile([1, 2 * N_VH], f32, name="ab_row")
            nc.scalar.copy(ab_row[:], ps_ab[:])

            # qk / v / z tiles; first two at 256-wide for an early PE start
            ps_qk = puno([1, 512], "ps_qk")
            ps_v = puno([1, 512], "ps_v")
            ps_z = puno([1, 512], "ps_z")
            jtiles = [(0, 256, ps_qk, 0), (256, 256, ps_qk, 256),
                      (512, 512, ps_v, 0), (1024, 512, ps_z, 0)]
            for j0, jw, ps, off in jtiles:
                wt = wp.tile([128, 16, jw], f32r, tag="w", name="w1t",
                             padded_shape=[128, 16, 512])
                nc.sync.dma_start(out=wt[:], in_=w1[:, :, j0:j0 + jw])
                for c in range(16):
                    nc.tensor.matmul(ps[0:1, off:off + jw],
                                     lhsT=h_r[:, c:c + 1], rhs=wt[:, c, :],
                                     start=(c == 0), stop=(c == 15))

            qkv_row = sp.tile([1, CH], f32, name="qkv_row")
            nc.scalar.copy(qkv_row[0:1, 0:512], ps_qk[:])
            nc.vector.tensor_copy(qkv_row[0:1, 512:1024], ps_v[:])

            # ---- decay / beta (from a,b rows) ----
            beta = sp.tile([1, N_VH], f32, name="beta")
            nc.scalar.activation(beta[:], ab_row[0:1, N_VH:2 * N_VH], AF.Exp,
                                 scale=-1.0)
            nc.vector.tensor_scalar_add(beta[:], beta[:], 1.0)
            nc.vector.reciprocal(beta[:], beta[:])
            spin = sp.tile([1, N_VH], f32, name="spin")
            nc.vector.tensor_tensor(spin[:], ab_row[0:1, 0:N_VH],
                                    scal_s[0:1, 4:8], ALU.add)
            sp_t = sp.tile([1, N_VH], f32, name="sp_t")
            nc.scalar.activation(sp_t[:], spin[:], AF.Exp)
            nc.vector.tensor_scalar_add(sp_t[:], sp_t[:], 1.0)
            nc.scalar.activation(sp_t[:], sp_t[:], AF.Ln)
            eA = sp.tile([1, N_VH], f32, name="eA")
            nc.scalar.activation(eA[:], scal_s[0:1, 0:4], AF.Exp)
            gabs = sp.tile([1, N_VH], f32, name="gabs")
            nc.vector.tensor_tensor(gabs[:], eA[:], sp_t[:], ALU.mult)
            decay = sp.tile([1, N_VH], f32, name="decay")
            nc.scalar.activation(decay[:], gabs[:], AF.Exp, scale=-1.0)
            ps_db = puno([128, N_VH], "ps_db")
            nc.tensor.matmul(ps_db[:], lhsT=ones_r[:], rhs=decay[:],
                             start=True, stop=True)
            decay_bc = sp.tile([128, N_VH], f32, name="decay_bc")
            nc.scalar.copy(decay_bc[:], ps_db[:])
            decayed = sp.tile([128, N_VH * VD], f32, name="decayed")
            for vh in range(N_VH):
                nc.vector.tensor_scalar(
                    decayed[:, vh * VD:(vh + 1) * VD],
                    ssm_s[:, vh * VD:(vh + 1) * VD],
                    decay_bc[:, vh:vh + 1], None, ALU.mult)

            # ---- mixed_qkv row -> columns (one head per column) ----
            ps_qc = puno([128, 8], "ps_qc")
            for i in range(8):
                nc.tensor.transpose(ps_qc[:, i:i + 1],
                                    qkv_row[0:1, i * 128:(i + 1) * 128],
                                    ones11[:])
            qkv_col = sp.tile([128, 8], f32, name="qkv_col")
            nc.vector.tensor_copy(qkv_col[:], ps_qc[:])
            nc.scalar.dma_start(out=qkv_out[:], in_=qkv_col[:])

            # ---- conv step + silu, column space ----
            convs = sp.tile([128, 8], f32, name="convs")
            nc.vector.tensor_tensor(convs[:], qkv_col[:], cwlast_s[:], ALU.mult)
            nc.vector.tensor_tensor(convs[:], convs[:], convp_s[:], ALU.add)
            conv_c = sp.tile([128, 8], f32, name="conv_c")
            nc.scalar.activation(conv_c[:], convs[:], AF.Exp, scale=-1.0)
            nc.vector.tensor_scalar_add(conv_c[:], conv_c[:], 1.0)
            nc.vector.reciprocal(conv_c[:], conv_c[:])
            nc.vector.tensor_tensor(conv_c[:], conv_c[:], convs[:], ALU.mult)

            # ---- l2-normalize q,k columns (cols 0,1=q; 2,3=k) ----
            sq4 = sp.tile([128, 4], f32, name="sq4")
            nc.vector.tensor_tensor(sq4[:], conv_c[:, 0:4], conv_c[:, 0:4],
                                    ALU.mult)
            ps_n4 = puno([1, 4], "ps_n4")
            nc.tensor.matmul(ps_n4[:], lhsT=ones_c[:], rhs=sq4[:],
                             start=True, stop=True)
            ss4 = sp.tile([1, 4], f32, name="ss4")
            nc.vector.tensor_scalar_max(ss4[:], ps_n4[:], 1e-24)
            nc.scalar.activation(ss4[:], ss4[:], AF.Ln)
            nc.scalar.activation(ss4[:], ss4[:], AF.Exp, scale=-0.5)
            nc.vector.tensor_tensor(ss4[:], ss4[:], qmask_s[:], ALU.mult)
            ps_n4b = puno([128, 4], "ps_n4b")
            nc.tensor.matmul(ps_n4b[:], lhsT=ones_r[:], rhs=ss4[:],
                             start=True, stop=True)
            qkn = sp.tile([128, 4], f32, name="qkn")
            nc.vector.tensor_tensor(qkn[:], conv_c[:, 0:4], ps_n4b[:], ALU.mult)

            # ---- delta rule, column space ----
            # Sk_col[:, vh] = decayed_blk(vh).T @ k_col(vh//2)
            ps_skc = puno([128, 4], "ps_skc")
            for vh in range(N_VH):
                nc.tensor.matmul(ps_skc[:, vh:vh + 1],
                                 lhsT=decayed[:, vh * VD:(vh + 1) * VD],
                                 rhs=qkn[:, 2 + vh // 2:3 + vh // 2],
                                 start=True, stop=True)
            ps_bb = puno([128, 4], "ps_bb")
            nc.tensor.matmul(ps_bb[:], lhsT=ones_r[:], rhs=beta[:],
                             start=True, stop=True)
            delta_c = sp.tile([128, 4], f32, name="delta_c")
            nc.vector.tensor_tensor(delta_c[:], conv_c[:, 4:8], ps_skc[:],
                                    ALU.subtract)
            nc.vector.tensor_tensor(delta_c[:], delta_c[:], ps_bb[:], ALU.mult)
            # rows needed for the rank-1 outer products
            ps_dr = puno([1, 512], "ps_dr")
            for i in range(4):
                nc.tensor.transpose(ps_dr[0:1, i * 128:(i + 1) * 128],
                                    delta_c[:, i:i + 1], id_s[:])
            delta_r = sp.tile([1, 512], f32, name="delta_r")
            nc.scalar.copy(delta_r[:], ps_dr[:])
            ps_kr = puno([1, 256], "ps_kr")
            for i in range(2):
                nc.tensor.transpose(ps_kr[0:1, i * 128:(i + 1) * 128],
                                    qkn[:, 2 + i:3 + i], id_s[:])
            k_row = sp.tile([1, 256], f32, name="k_row")
            nc.vector.tensor_copy(k_row[:], ps_kr[:])

            ps_ssm = puno([128, 512], "ps_ssm")
            for vh in range(N_VH):
                nc.tensor.matmul(ps_ssm[:, vh * 128:(vh + 1) * 128],
                                 lhsT=k_row[0:1, (vh // 2) * 128:
                                            (vh // 2 + 1) * 128],
                                 rhs=delta_r[0:1, vh * 128:(vh + 1) * 128],
                                 start=True, stop=True)
            new_ssm = sp.tile([128, 512], f32, name="new_ssm")
            nc.vector.tensor_tensor(new_ssm[:], ps_ssm[:], decayed[:], ALU.add)
            nc.scalar.dma_start(out=ssm_out[:], in_=new_ssm[:])

            # y_col[:, vh] = new_ssm_blk(vh).T @ q_col(vh//2)
            ps_yc = puno([128, 4], "ps_yc")
            for vh in range(N_VH):
                nc.tensor.matmul(ps_yc[:, vh:vh + 1],
                                 lhsT=new_ssm[:, vh * 128:(vh + 1) * 128],
                                 rhs=qkn[:, vh // 2:vh // 2 + 1],
                                 start=True, stop=True)
            y_col = sp.tile([128, 4], f32, name="y_col")
            nc.scalar.copy(y_col[:], ps_yc[:])

            # ---- gated rmsnorm (col) ----
            ysq = sp.tile([128, 4], f32, name="ysq")
            nc.vector.tensor_tensor(ysq[:], y_col[:], y_col[:], ALU.mult)
            ps_y4 = puno([1, 4], "ps_y4")
            nc.tensor.matmul(ps_y4[:], lhsT=ones_c[:], rhs=ysq[:],
                             start=True, stop=True)
            ys4 = sp.tile([1, 4], f32, name="ys4")
            nc.scalar.activation(ys4[:], ps_y4[:], AF.Ln, scale=1.0 / VD,
                                 bias=eps1[:])
            nc.scalar.activation(ys4[:], ys4[:], AF.Exp, scale=-0.5)
            ps_y4b = puno([128, 4], "ps_y4b")
            nc.tensor.matmul(ps_y4b[:], lhsT=ones_r[:], rhs=ys4[:],
                             start=True, stop=True)
            nc.vector.tensor_tensor(y_col[:], y_col[:], ps_y4b[:], ALU.mult)
            nc.vector.tensor_scalar(y_col[:], y_col[:], nw_s[:], None, ALU.mult)

            # ---- silu(z) gate (col) ----
            z_row = sp.tile([1, 512], f32, name="z_row")
            nc.scalar.copy(z_row[:], ps_z[:])
            ps_zc = puno([128, 4], "ps_zc")
            for i in range(4):
                nc.tensor.transpose(ps_zc[:, i:i + 1],
                                    z_row[0:1, i * 128:(i + 1) * 128],
                                    ones11[:])
            z_col = sp.tile([128, 4], f32, name="z_col")
            nc.vector.tensor_copy(z_col[:], ps_zc[:])
            zs_col = sp.tile([128, 4], f32, name="zs_col")
            nc.scalar.activation(zs_col[:], z_col[:], AF.Exp, scale=-1.0)
            nc.vector.tensor_scalar_add(zs_col[:], zs_col[:], 1.0)
            nc.vector.reciprocal(zs_col[:], zs_col[:])
            nc.vector.tensor_tensor(zs_col[:], zs_col[:], z_col[:], ALU.mult)
            yo_col = sp.tile([128, 4], f32r, name="yo_col")
            nc.vector.tensor_tensor(yo_col[:], y_col[:], zs_col[:], ALU.mult)

            # ---- out_proj partial -> [1,2048] row ----
            ps_attn = pp.tile([1, H], f32, tag="big4", bufs=1, name="ps_attn")
            wot = wp.tile([128, 4, H], f32r, tag="w", name="wot")
            nc.sync.dma_start(out=wot[:], in_=wo[:])
            for sub in range(4):
                off = sub * 512
                for c in range(4):
                    nc.tensor.matmul(
                        ps_attn[0:1, off:off + 512],
                        lhsT=yo_col[:, c:c + 1],
                        rhs=wot[:, c, off:off + 512],
                        start=(c == 0), stop=(c == 3))
            attn_row = sp.tile([1, H], f32, name="attn_row")
            nc.scalar.copy(attn_row[0:1, 0:512], ps_attn[0:1, 0:512])
            nc.vector.tensor_copy(attn_row[0:1, 512:1024], ps_attn[0:1, 512:1024])
            nc.scalar.copy(attn_row[0:1, 1024:1536], ps_attn[0:1, 1024:1536])
            nc.vector.tensor_copy(attn_row[0:1, 1536:2048],
                                  ps_attn[0:1, 1536:2048])
            ps_ac = puno([128, 16], "ps_ac")
            for i in range(16):
                nc.tensor.transpose(ps_ac[:, i:i + 1],
                                    attn_row[0:1, i * 128:(i + 1) * 128],
                                    ones11[:])
            ac_col = sp.tile([128, 16], f32, name="ac_col")
            nc.vector.tensor_copy(ac_col[:], ps_ac[:])

            # ---- AllReduce attn partials (column layout) ----
            ar_in = dp.tile([128, 16], f32, name="ar_in")
            ar_out = dp.tile([128, 16], f32, name="ar_out")
            nc.scalar.dma_start(out=ar_in[:], in_=ac_col[:])
            nc.gpsimd.collective_compute(
                "AllReduce", ALU.add, ins=[ar_in.opt()], outs=[ar_out.opt()],
                replica_groups=[list(range(NC))])
            ar_col = sp.tile([128, 16], f32, name="ar_col")
            nc.scalar.dma_start(out=ar_col[:], in_=ar_out[:])

            # ---- residual + rms2 (col) ----
            nc.vector.tensor_tensor(ar_col[:], ar_col[:], xc_s[:], ALU.add)
            nc.scalar.dma_start(out=x1_out[:], in_=ar_col[:])
            sq2 = sp.tile([128, 16], f32, name="sq2")
            x2acc = sp.tile([128, 1], f32, name="x2acc")
            nc.scalar.activation(sq2[:], ar_col[:], AF.Square,
                                 accum_out=x2acc[:])
            ps_s2 = puno([1, 1], "ps_s2")
            nc.tensor.matmul(ps_s2[:], lhsT=ones_c[:], rhs=x2acc[:],
                             start=True, stop=True)
            s2p = sp.tile([1, 1], f32, name="s2p")
            nc.scalar.activation(s2p[:], ps_s2[:], AF.Ln, scale=1.0 / H,
                                 bias=eps1[:])
            s2 = sp.tile([1, 1], f32, name="s2")
            nc.scalar.activation(s2[:], s2p[:], AF.Exp, scale=-0.5)
            ps_s2b = puno([128, 1], "ps_s2b")
            nc.tensor.matmul(ps_s2b[:], lhsT=ones_r[:], rhs=s2[:],
                             start=True, stop=True)
            s2b = sp.tile([128, 1], f32, name="s2b")
            nc.scalar.copy(s2b[:], ps_s2b[:])
            h2 = sp.tile([128, 16], f32, name="h2")
            nc.vector.tensor_scalar(h2[:], ar_col[:], s2b[:], None, ALU.mult)
            h2c = sp.tile([128, 16], f32r, name="h2c")
            nc.vector.tensor_tensor(h2c[:], h2[:], w2f_s[:], ALU.mult)

            # ---- FFN gate/up matvecs ----
            ps_gu = pp.tile([1, 2 * IC], f32, tag="big4", bufs=1, name="ps_gu")
            for jt in range(2):
                wgt = wp.tile([128, 16, 512], f32r, tag="w", name="wgt")
                nc.sync.dma_start(out=wgt[:], in_=wg[:, :, jt * 512:(jt + 1) * 512])
                for c in range(16):
                    nc.tensor.matmul(ps_gu[0:1, jt * 512:(jt + 1) * 512],
                                     lhsT=h2c[:, c:c + 1], rhs=wgt[:, c, :],
                                     start=(c == 0), stop=(c == 15))
                wut = wp.tile([128, 16, 512], f32r, tag="w", name="wut")
                nc.sync.dma_start(out=wut[:], in_=wu[:, :, jt * 512:(jt + 1) * 512])
                for c in range(16):
                    nc.tensor.matmul(ps_gu[0:1, IC + jt * 512:IC + (jt + 1) * 512],
                                     lhsT=h2c[:, c:c + 1], rhs=wut[:, c, :],
                                     start=(c == 0), stop=(c == 15))
            # silu(g)*u in 256-chunks, alternating engines to pipeline
            s_row = sp.tile([1, IC], f32, name="s_row")
            for jt in range(4):
                gsl = s_row[0:1, jt * 256:(jt + 1) * 256]
                gch = ps_gu[0:1, jt * 256:(jt + 1) * 256]
                uch = ps_gu[0:1, IC + jt * 256:IC + (jt + 1) * 256]
                gs_row = sp.tile([1, 256], f32, tag="gs_row", name="gs_row",
                                 bufs=2)
                nc.scalar.activation(gs_row[:], gch, AF.Exp, scale=-1.0)
                nc.vector.tensor_scalar_add(gs_row[:], gs_row[:], 1.0)
                nc.vector.reciprocal(gs_row[:], gs_row[:])
                nc.vector.tensor_tensor(gs_row[:], gs_row[:], gch, ALU.mult)
                nc.vector.tensor_tensor(gsl, gs_row[:], uch, ALU.mult)
            ps_sT = puno([128, 8], "ps_sT")
            for i in range(8):
                nc.tensor.transpose(ps_sT[:, i:i + 1],
                                    s_row[0:1, i * 128:(i + 1) * 128],
                                    ones11[:])
            s_col = sp.tile([128, 8], f32r, name="s_col")
            nc.scalar.copy(s_col[:], ps_sT[:])

            # ---- FFN down matvec -> partial [1,2048] ----
            ps_ffn = pp.tile([1, H], f32, tag="big4", bufs=1, name="ps_ffn")
            for jt in range(2):
                wdt = wp.tile([128, 8, 1024], f32r, tag="w", name="wdt")
                nc.sync.dma_start(out=wdt[:], in_=wd[:, :, jt * 1024:(jt + 1) * 1024])
                for sub in range(2):
                    off = jt * 1024 + sub * 512
                    for c in range(8):
                        nc.tensor.matmul(ps_ffn[0:1, off:off + 512],
                                         lhsT=s_col[:, c:c + 1],
                                         rhs=wdt[:, c, sub * 512:(sub + 1) * 512],
                                         start=(c == 0), stop=(c == 7))
            ffn_row = sp.tile([1, H], f32, name="ffn_row")
            nc.scalar.copy(ffn_row[0:1, 0:512], ps_ffn[0:1, 0:512])
            nc.vector.tensor_copy(ffn_row[0:1, 512:1024], ps_ffn[0:1, 512:1024])
            nc.scalar.copy(ffn_row[0:1, 1024:1536], ps_ffn[0:1, 1024:1536])
            nc.vector.tensor_copy(ffn_row[0:1, 1536:2048],
                                  ps_ffn[0:1, 1536:2048])
            nc.scalar.dma_start(out=ffn_out[:], in_=ffn_row[:])

    nc.compile()
    return nc


_CACHE = {}


def _get_program():
    if "nc" not in _CACHE:
        _CACHE["nc"] = build_program()
    return _CACHE["nc"]


def _shard_inputs(x, conv_state, ssm_state, in_proj_w, out_proj_w, conv_weight,
                  A_log, dt_bias, norm_weight, rms1_w, rms2_w, gate_w, up_w,
                  down_w):
    """Pack + shard full inputs into per-core input maps."""
    f = np.float32
    in_maps = []
    ch_idx_all = []
    for c in range(NC):
        q_rows = np.arange(256 * c, 256 * (c + 1))
        k_rows = 2048 + q_rows
        v_rows = np.arange(4096 + 512 * c, 4096 + 512 * (c + 1))
        z_rows = np.arange(QKV + 512 * c, QKV + 512 * (c + 1))
        a_rows = np.arange(QKV + TV + 4 * c, QKV + TV + 4 * (c + 1))
        b_rows = a_rows + NVH
        rows1 = np.concatenate([q_rows, k_rows, v_rows, z_rows, a_rows, b_rows])
        ch_idx = np.concatenate([q_rows, k_rows, v_rows])  # conv channels
        ch_idx_all.append(ch_idx)
        vs = slice(512 * c, 512 * (c + 1))
        ics = slice(IC * c, IC * (c + 1))
        vh = slice(4 * c, 4 * (c + 1))

        cs = conv_state[ch_idx]          # [1024, 4]
        cw = conv_weight[ch_idx]         # [1024, 4]
        in_maps.append({
            "w1": _pack_T(in_proj_w[rows1]),
            "wo": _pack_T(out_proj_w[:, vs]),
            "wg": _pack_T(gate_w[ics]),
            "wu": _pack_T(up_w[ics]),
            "wd": _pack_T(down_w[:, ics]),
            "xc": _colmaj(x[0]).astype(f),
            "w1f": _colmaj(1.0 + rms1_w).astype(f),
            "w2f": _colmaj(1.0 + rms2_w).astype(f),
            "convp": _colmaj(np.sum(cs[:, 1:4] * cw[:, 0:3], axis=1)).astype(f),
            "cwlast": _colmaj(cw[:, 3]).astype(f),
            "ssm": np.concatenate([ssm_state[i] for i in range(4 * c, 4 * c + 4)],
                                  axis=1).astype(f),
            "scal": np.concatenate([A_log[vh], dt_bias[vh]])[None, :].astype(f),
            "qmask": np.array([[INV_SQRT_KD, INV_SQRT_KD, 1.0, 1.0]], f),
            "nwc": norm_weight[:, None].astype(f),
            "ident": np.eye(128, dtype=f),
        })
    return in_maps, ch_idx_all


def kernel(**inputs):
    inputs = {k: np.asarray(v) for k, v in inputs.items()}
    nc = _get_program()
    in_maps, ch_idx_all = _shard_inputs(**inputs)
    res = run_bass_kernel_spmd(nc, in_maps, core_ids=list(range(NC))).results

    # ---- unshard ----
    x_final = res[0]["x1_out"].T.reshape(1, H).astype(np.float64)
    for c in range(NC):
        x_final = x_final + res[c]["ffn_out"].astype(np.float64)
    x_final = x_final.astype(np.float32)

    conv_state = inputs["conv_state"]
    new_conv = np.empty((QKV, KC), np.float32)
    new_conv[:, 0:3] = conv_state[:, 1:4]
    for c in range(NC):
        new_conv[ch_idx_all[c], 3] = res[c]["qkv_out"].T.reshape(-1)

    new_ssm = np.empty((NVH, KD, VD), np.float32)
    for c in range(NC):
        blk = res[c]["ssm_out"].reshape(KD, N_VH, VD).transpose(1, 0, 2)
        new_ssm[4 * c:4 * (c + 1)] = blk

    return x_final, new_conv, new_ssm


# revision 9
# speedup vs baseline: 1.5316x; 1.2300x over previous
"""DeltaNet decode-step layer on 8 TRN2 NeuronCores (Bass/Tile).

Sharding (tensor-parallel over the 32 value heads + FFN intermediate):
  - core c owns value heads [4c, 4c+4) and key heads [2c, 2c+2)
  - in_proj rows / conv channels / z rows / a,b rows for those heads
  - out_proj columns for those heads (partial attn output, AllReduce'd)
  - FFN intermediate rows [1024c, 1024c+1024) (partial down output,
    summed on the host during unshard)

Weights are pre-transposed + packed on the host so every big matvec
streams the weight as the matmul *moving* operand in fp32r at
1 column/cycle:  pack[p, c, j] = W.T[c*128+p, j].

Elementwise chains run in partition-parallel "column" layouts
([128, nheads] tiles, one head per free column) — single-partition
[1, N] ops process only one lane/cycle and would dominate the
critical path otherwise.  PE transposes move rows<->columns.

A tiny dummy AllGather is issued first so the once-per-NEFF collective
entry barrier overlaps the in_proj weight streaming instead of the
real AllReduce.
"""
import numpy as np

import concourse.bacc as bacc
import concourse.mybir as mybir
import concourse.tile as tile
from concourse.bass_utils import run_bass_kernel_spmd

dt = mybir.dt
AF = mybir.ActivationFunctionType
ALU = mybir.AluOpType

H, I, QKV, TV = 2048, 8192, 8192, 4096
NVH, NKH, KD, VD, KC = 32, 16, 128, 128, 4
NC = 8
N_VH, N_KH = NVH // NC, NKH // NC          # 4 value heads, 2 key heads/core
CH = N_KH * KD * 2 + N_VH * VD             # 1024 conv channels/core
NZ = N_VH * VD                             # 512 z rows/core
R1 = CH + NZ + 2 * N_VH                    # 1544 in_proj rows/core
IC = I // NC                               # 1024 ffn rows/core
EPS = 1e-6
INV_SQRT_KD = float(KD) ** -0.5


def _pack_T(w_rows):
    """[J, H'] weight rows -> fp32 [128, H'/128, J] moving-operand pack."""
    t = np.ascontiguousarray(w_rows.T)                 # [H', J]
    hp, j = t.shape
    return np.ascontiguousarray(
        t.reshape(hp // 128, 128, j).transpose(1, 0, 2))


def _colmaj(v):
    """[N]-vector -> [128, N/128] column-major tile (elem i -> [i%128, i//128])."""
    return np.ascontiguousarray(v.reshape(-1, 128).T)


def build_program():
    nc = bacc.Bacc("TRN2", target_bir_lowering=False, debug=False,
                   num_devices=NC)
    f32, f32r = dt.float32, dt.float32r

    # ---- per-core DRAM inputs ----
    w1 = nc.dram_tensor("w1", [128, 16, R1], f32r, kind="ExternalInput")
    wo = nc.dram_tensor("wo", [128, 4, H], f32r, kind="ExternalInput")
    wg = nc.dram_tensor("wg", [128, 16, IC], f32r, kind="ExternalInput")
    wu = nc.dram_tensor("wu", [128, 16, IC], f32r, kind="ExternalInput")
    wd = nc.dram_tensor("wd", [128, 8, H], f32r, kind="ExternalInput")
    xc = nc.dram_tensor("xc", [128, 16], f32, kind="ExternalInput")
    w1f = nc.dram_tensor("w1f", [128, 16], f32, kind="ExternalInput")
    w2f = nc.dram_tensor("w2f", [128, 16], f32, kind="ExternalInput")
    convp = nc.dram_tensor("convp", [128, 8], f32, kind="ExternalInput")
    cwlast = nc.dram_tensor("cwlast", [128, 8], f32, kind="ExternalInput")
    ssm = nc.dram_tensor("ssm", [128, N_VH * VD], f32, kind="ExternalInput")
    scal = nc.dram_tensor("scal", [1, 8], f32, kind="ExternalInput")
    qmask = nc.dram_tensor("qmask", [1, 4], f32, kind="ExternalInput")
    nwc = nc.dram_tensor("nwc", [128, 1], f32, kind="ExternalInput")
    ident = nc.dram_tensor("ident", [128, 128], f32, kind="ExternalInput")

    # ---- per-core DRAM outputs ----
    qkv_out = nc.dram_tensor("qkv_out", [128, 8], f32, kind="ExternalOutput")
    ssm_out = nc.dram_tensor("ssm_out", [128, N_VH * VD], f32,
                             kind="ExternalOutput")
    x1_out = nc.dram_tensor("x1_out", [128, 16], f32, kind="ExternalOutput")
    ffn_out = nc.dram_tensor("ffn_out", [1, H], f32, kind="ExternalOutput")
    dbg_out = nc.dram_tensor("dbg_out", [NC, 4], f32, kind="ExternalOutput")

    with tile.TileContext(nc, trace_sim=False) as tc:
        with tc.tile_pool(name="wp", bufs=8) as wp, \
             tc.tile_pool(name="sp", bufs=1) as sp, \
             tc.tile_pool(name="pp", bufs=1, space="PSUM") as pp, \
             tc.tile_pool(name="dp", bufs=1, space="DRAM") as dp:

            def puno(shape, name):
                return pp.tile(shape, f32, tag="uno", bufs=4, name=name)

            # ---- dummy collective: absorb the once-per-NEFF barrier ----
            dz = sp.tile([1, 4], f32, name="dz")
            nc.vector.memset(dz[:], 1.0)
            d_in = dp.tile([1, 4], f32, name="d_in")
            d_out = dp.tile([NC, 4], f32, name="d_out")
            nc.scalar.dma_start(out=d_in[:], in_=dz[:])
            nc.gpsimd.collective_compute(
                "AllGather", ALU.bypass, ins=[d_in.opt()], outs=[d_out.opt()],
                replica_groups=[list(range(NC))])
            dzo = sp.tile([NC, 4], f32, name="dzo")
            nc.gpsimd.dma_start(out=dzo[:], in_=d_out[:])
            nc.gpsimd.dma_start(out=dbg_out[:], in_=dzo[:])

            # ---- small input loads (ACT HWDGE ring) ----
            def load(name, shape, src):
                t = sp.tile(shape, f32, name=name)
                nc.scalar.dma_start(out=t[:], in_=src[:])
                return t
            xc_s = load("xc_s", [128, 16], xc)
            w1f_s = load("w1f_s", [128, 16], w1f)
            scal_s = load("scal_s", [1, 8], scal)
            qmask_s = load("qmask_s", [1, 4], qmask)
            convp_s = load("convp_s", [128, 8], convp)
            cwlast_s = load("cwlast_s", [128, 8], cwlast)
            nw_s = load("nw_s", [128, 1], nwc)
            ssm_s = load("ssm_s", [128, N_VH * VD], ssm)
            w2f_s = load("w2f_s", [128, 16], w2f)
            id_s = load("id_s", [128, 128], ident)

            ones11 = sp.tile([1, 1], f32, name="ones11")
            nc.vector.memset(ones11[:], 1.0)
            ones_r = sp.tile([1, 128], f32, name="ones_r")
            nc.vector.memset(ones_r[:], 1.0)
            ones_c = sp.tile([128, 1], f32, name="ones_c")
            nc.vector.memset(ones_c[:], 1.0)
            eps1 = sp.tile([1, 1], f32, name="eps1")
            nc.vector.memset(eps1[:], EPS)

            # ---- stage A: h = rms(x) * (1 + rms1_w), column layout ----
            sqA = sp.tile([128, 16], f32, name="sqA")
            xacc = sp.tile([128, 1], f32, name="xacc")
            nc.scalar.activation(sqA[:], xc_s[:], AF.Square, accum_out=xacc[:])
            ps_s1 = puno([1, 1], "ps_s1")
            nc.tensor.matmul(ps_s1[:], lhsT=ones_c[:], rhs=xacc[:],
                             start=True, stop=True)
            s1p = sp.tile([1, 1], f32, name="s1p")
            nc.scalar.activation(s1p[:], ps_s1[:], AF.Ln,
                                 scale=1.0 / H, bias=eps1[:])
            s1 = sp.tile([1, 1], f32, name="s1")
            nc.scalar.activation(s1[:], s1p[:], AF.Exp, scale=-0.5)
            ps_s1b = puno([128, 1], "ps_s1b")
            nc.tensor.matmul(ps_s1b[:], lhsT=ones_r[:], rhs=s1[:],
                             start=True, stop=True)
            s1b = sp.tile([128, 1], f32, name="s1b")
            nc.scalar.copy(s1b[:], ps_s1b[:])
            h1 = sp.tile([128, 16], f32, name="h1")
            nc.vector.tensor_scalar(h1[:], xc_s[:], s1b[:], None, ALU.mult)
            h_r = sp.tile([128, 16], f32r, name="h_r")
            nc.vector.tensor_tensor(h_r[:], h1[:], w1f_s[:], ALU.mult)

            # ---- in_proj matvecs (fp32r, weight moving) ----
            # a/b rows first: decay/beta computed early off the critical path
            w1ab = wp.tile([128, 16, 2 * N_VH], f32r, tag="w", name="w1ab")
            nc.sync.dma_start(out=w1ab[:], in_=w1[:, :, CH + NZ:R1])
            ps_ab = puno([1, 2 * N_VH], "ps_ab")
            for c in range(16):
                nc.tensor.matmul(ps_ab[:], lhsT=h_r[:, c:c + 1],
                                 rhs=w1ab[:, c, :],
                                 start=(c == 0), stop=(c == 15))
            ab_row = sp.tile([1, 2 * N_VH], f32, name="ab_row")
            nc.scalar.copy(ab_row[:], ps_ab[:])

            # qk / v / z tiles; first two at 256-wide for an early PE start
            ps_qk = puno([1, 512], "ps_qk")
            ps_v = puno([1, 512], "ps_v")
            ps_z = puno([1, 512], "ps_z")
            jtiles = [(0, ps_qk, 0), (256, ps_qk, 256),
                      (512, ps_v, 0), (768, ps_v, 256),
                      (1024, ps_z, 0), (1280, ps_z, 256)]
            for j0, ps, off in jtiles:
                wt = wp.tile([128, 16, 256], f32r, tag="w", name="w1t")
                nc.sync.dma_start(out=wt[:], in_=w1[:, :, j0:j0 + 256])
                for c in range(16):
                    nc.tensor.matmul(ps[0:1, off:off + 256],
                                     lhsT=h_r[:, c:c + 1], rhs=wt[:, c, :],
                                     start=(c == 0), stop=(c == 15))

            qkv_row = sp.tile([1, CH], f32, name="qkv_row")
            nc.scalar.copy(qkv_row[0:1, 0:512], ps_qk[:])
            nc.vector.tensor_copy(qkv_row[0:1, 512:1024], ps_v[:])

            # ---- decay / beta (from a,b rows) ----
            beta = sp.tile([1, N_VH], f32, name="beta")
            nc.scalar.activation(beta[:], ab_row[0:1, N_VH:2 * N_VH], AF.Exp,
                                 scale=-1.0)
            nc.vector.tensor_scalar_add(beta[:], beta[:], 1.0)
            nc.vector.reciprocal(beta[:], beta[:])
            spin = sp.tile([1, N_VH], f32, name="spin")
            nc.vector.tensor_tensor(spin[:], ab_row[0:1, 0:N_VH],
                                    scal_s[0:1, 4:8], ALU.add)
            sp_t = sp.tile([1, N_VH], f32, name="sp_t")
            nc.scalar.activation(sp_t[:], spin[:], AF.Exp)
            nc.vector.tensor_scalar_add(sp_t[:], sp_t[:], 1.0)
            nc.scalar.activation(sp_t[:], sp_t[:], AF.Ln)
            eA = sp.tile([1, N_VH], f32, name="eA")
            nc.scalar.activation(eA[:], scal_s[0:1, 0:4], AF.Exp)
            gabs = sp.tile([1, N_VH], f32, name="gabs")
            nc.vector.tensor_tensor(gabs[:], eA[:], sp_t[:], ALU.mult)
            decay = sp.tile([1, N_VH], f32, name="decay")
            nc.scalar.activation(decay[:], gabs[:], AF.Exp, scale=-1.0)
            ps_db = puno([128, N_VH], "ps_db")
            nc.tensor.matmul(ps_db[:], lhsT=ones_r[:], rhs=decay[:],
                             start=True, stop=True)
            decay_bc = sp.tile([128, N_VH], f32, name="decay_bc")
            nc.scalar.copy(decay_bc[:], ps_db[:])
            decayed = sp.tile([128, N_VH * VD], f32, name="decayed")
            for vh in range(N_VH):
                nc.vector.tensor_scalar(
                    decayed[:, vh * VD:(vh + 1) * VD],
                    ssm_s[:, vh * VD:(vh + 1) * VD],
                    decay_bc[:, vh:vh + 1], None, ALU.mult)

            # ---- mixed_qkv row -> columns (one head per column) ----
            ps_qc = puno([128, 8], "ps_qc")
            for i in range(8):
                nc.tensor.transpose(ps_qc[:, i:i + 1],
                                    qkv_row[0:1, i * 128:(i + 1) * 128],
                                    ones11[:])
            qkv_col = sp.tile([128, 8], f32, name="qkv_col")
            nc.vector.tensor_copy(qkv_col[:], ps_qc[:])
            nc.scalar.dma_start(out=qkv_out[:], in_=qkv_col[:])

            # ---- conv step + silu, column space ----
            convs = sp.tile([128, 8], f32, name="convs")
            nc.vector.tensor_tensor(convs[:], qkv_col[:], cwlast_s[:], ALU.mult)
            nc.vector.tensor_tensor(convs[:], convs[:], convp_s[:], ALU.add)
            conv_c = sp.tile([128, 8], f32, name="conv_c")
            nc.scalar.activation(conv_c[:], convs[:], AF.Exp, scale=-1.0)
            nc.vector.tensor_scalar_add(conv_c[:], conv_c[:], 1.0)
            nc.vector.reciprocal(conv_c[:], conv_c[:])
            nc.vector.tensor_tensor(conv_c[:], conv_c[:], convs[:], ALU.mult)

            # ---- l2-normalize q,k columns (cols 0,1=q; 2,3=k) ----
            sq4 = sp.tile([128, 4], f32, name="sq4")
            nc.vector.tensor_tensor(sq4[:], conv_c[:, 0:4], conv_c[:, 0:4],
                                    ALU.mult)
            ps_n4 = puno([1, 4], "ps_n4")
            nc.tensor.matmul(ps_n4[:], lhsT=ones_c[:], rhs=sq4[:],
                             start=True, stop=True)
            ss4 = sp.tile([1, 4], f32, name="ss4")
            nc.vector.tensor_scalar_max(ss4[:], ps_n4[:], 1e-24)
            nc.scalar.activation(ss4[:], ss4[:], AF.Ln)
            nc.scalar.activation(ss4[:], ss4[:], AF.Exp, scale=-0.5)
            nc.vector.tensor_tensor(ss4[:], ss4[:], qmask_s[:], ALU.mult)
            ps_n4b = puno([128, 4], "ps_n4b")
            nc.tensor.matmul(ps_n4b[:], lhsT=ones_r[:], rhs=ss4[:],
                             start=True, stop=True)
            qkn = sp.tile([128, 4], f32, name="qkn")
            nc.vector.tensor_tensor(qkn[:], conv_c[:, 0:4], ps_n4b[:], ALU.mult)

            # ---- delta rule, column space ----
            # Sk_col[:, vh] = decayed_blk(vh).T @ k_col(vh//2)
            ps_skc = puno([128, 4], "ps_skc")
            for vh in range(N_VH):
                nc.tensor.matmul(ps_skc[:, vh:vh + 1],
                                 lhsT=decayed[:, vh * VD:(vh + 1) * VD],
                                 rhs=qkn[:, 2 + vh // 2:3 + vh // 2],
                                 start=True, stop=True)
            ps_bb = puno([128, 4], "ps_bb")
            nc.tensor.matmul(ps_bb[:], lhsT=ones_r[:], rhs=beta[:],
                             start=True, stop=True)
            delta_c = sp.tile([128, 4], f32, name="delta_c")
            nc.vector.tensor_tensor(delta_c[:], conv_c[:, 4:8], ps_skc[:],
                                    ALU.subtract)
            nc.vector.tensor_tensor(delta_c[:], delta_c[:], ps_bb[:], ALU.mult)
            # rows needed for the rank-1 outer products
            ps_dr = puno([1, 512], "ps_dr")
            for i in range(4):
                nc.tensor.transpose(ps_dr[0:1, i * 128:(i + 1) * 128],
                                    delta_c[:, i:i + 1], id_s[:])
            delta_r = sp.tile([1, 512], f32, name="delta_r")
            nc.scalar.copy(delta_r[:], ps_dr[:])
            ps_kr = puno([1, 256], "ps_kr")
            for i in range(2):
                nc.tensor.transpose(ps_kr[0:1, i * 128:(i + 1) * 128],
                                    qkn[:, 2 + i:3 + i], id_s[:])
            k_row = sp.tile([1, 256], f32, name="k_row")
            nc.vector.tensor_copy(k_row[:], ps_kr[:])

            ps_ssm = puno([128, 512], "ps_ssm")
            for vh in range(N_VH):
                nc.tensor.matmul(ps_ssm[:, vh * 128:(vh + 1) * 128],
                                 lhsT=k_row[0:1, (vh // 2) * 128:
                                            (vh // 2 + 1) * 128],
                                 rhs=delta_r[0:1, vh * 128:(vh + 1) * 128],
                                 start=True, stop=True)
            new_ssm = sp.tile([128, 512], f32, name="new_ssm")
            nc.vector.tensor_tensor(new_ssm[:], ps_ssm[:], decayed[:], ALU.add)
            nc.scalar.dma_start(out=ssm_out[:], in_=new_ssm[:])

            # y_col[:, vh] = new_ssm_blk(vh).T @ q_col(vh//2)
            ps_yc = puno([128, 4], "ps_yc")
            for vh in range(N_VH):
                nc.tensor.matmul(ps_yc[:, vh:vh + 1],
                                 lhsT=new_ssm[:, vh * 128:(vh + 1) * 128],
                                 rhs=qkn[:, vh // 2:vh // 2 + 1],
                                 start=True, stop=True)
            y_col = sp.tile([128, 4], f32, name="y_col")
            nc.scalar.copy(y_col[:], ps_yc[:])

            # ---- gated rmsnorm (col) ----
            ysq = sp.tile([128, 4], f32, name="ysq")
            nc.vector.tensor_tensor(ysq[:], y_col[:], y_col[:], ALU.mult)
            ps_y4 = puno([1, 4], "ps_y4")
            nc.tensor.matmul(ps_y4[:], lhsT=ones_c[:], rhs=ysq[:],
                             start=True, stop=True)
            ys4 = sp.tile([1, 4], f32, name="ys4")
            nc.scalar.activation(ys4[:], ps_y4[:], AF.Ln, scale=1.0 / VD,
                                 bias=eps1[:])
            nc.scalar.activation(ys4[:], ys4[:], AF.Exp, scale=-0.5)
            ps_y4b = puno([128, 4], "ps_y4b")
            nc.tensor.matmul(ps_y4b[:], lhsT=ones_r[:], rhs=ys4[:],
                             start=True, stop=True)
            nc.vector.tensor_tensor(y_col[:], y_col[:], ps_y4b[:], ALU.mult)
            nc.vector.tensor_scalar(y_col[:], y_col[:], nw_s[:], None, ALU.mult)

            # ---- silu(z) gate (col) ----
            z_row = sp.tile([1, 512], f32, name="z_row")
            nc.scalar.copy(z_row[:], ps_z[:])
            ps_zc = puno([128, 4], "ps_zc")
            for i in range(4):
                nc.tensor.transpose(ps_zc[:, i:i + 1],
                                    z_row[0:1, i * 128:(i + 1) * 128],
                                    ones11[:])
            z_col = sp.tile([128, 4], f32, name="z_col")
            nc.vector.tensor_copy(z_col[:], ps_zc[:])
            zs_col = sp.tile([128, 4], f32, name="zs_col")
            nc.scalar.activation(zs_col[:], z_col[:], AF.Exp, scale=-1.0)
            nc.vector.tensor_scalar_add(zs_col[:], zs_col[:], 1.0)
            nc.vector.reciprocal(zs_col[:], zs_col[:])
            nc.vector.tensor_tensor(zs_col[:], zs_col[:], z_col[:], ALU.mult)
            yo_col = sp.tile([128, 4], f32r, name="yo_col")
            nc.vector.tensor_tensor(yo_col[:], y_col[:], zs_col[:], ALU.mult)

            # ---- out_proj partial -> [1,2048] row ----
            ps_attn = pp.tile([1, H], f32, tag="big4", bufs=1, name="ps_attn")
            for half in range(2):
                wot = wp.tile([128, 4, 1024], f32r, tag="w", name="wot")
                nc.sync.dma_start(out=wot[:],
                                  in_=wo[:, :, half * 1024:(half + 1) * 1024])
                for sub in range(2):
                    off = half * 1024 + sub * 512
                    for c in range(4):
                        nc.tensor.matmul(
                            ps_attn[0:1, off:off + 512],
                            lhsT=yo_col[:, c:c + 1],
                            rhs=wot[:, c, sub * 512:(sub + 1) * 512],
                            start=(c == 0), stop=(c == 3))
            attn_row = sp.tile([1, H], f32, name="attn_row")
            nc.scalar.copy(attn_row[0:1, 0:512], ps_attn[0:1, 0:512])
            nc.vector.tensor_copy(attn_row[0:1, 512:1024], ps_attn[0:1, 512:1024])
            nc.scalar.copy(attn_row[0:1, 1024:1536], ps_attn[0:1, 1024:1536])
            nc.vector.tensor_copy(attn_row[0:1, 1536:2048],
                                  ps_attn[0:1, 1536:2048])
            ps_ac = puno([128, 16], "ps_ac")
            for i in range(16):
                nc.tensor.transpose(ps_ac[:, i:i + 1],
                                    attn_row[0:1, i * 128:(i + 1) * 128],
                                    ones11[:])
            ac_col = sp.tile([128, 16], f32, name="ac_col")
            nc.vector.tensor_copy(ac_col[:], ps_ac[:])

            # ---- AllReduce attn partials (column layout) ----
            ar_in = dp.tile([128, 16], f32, name="ar_in")
            ar_out = dp.tile([128, 16], f32, name="ar_out")
            nc.scalar.dma_start(out=ar_in[:], in_=ac_col[:])
            nc.gpsimd.collective_compute(
                "AllReduce", ALU.add, ins=[ar_in.opt()], outs=[ar_out.opt()],
                replica_groups=[list(range(NC))])
            ar_col = sp.tile([128, 16], f32, name="ar_col")
            nc.scalar.dma_start(out=ar_col[:], in_=ar_out[:])

            # ---- residual + rms2 (col) ----
            nc.vector.tensor_tensor(ar_col[:], ar_col[:], xc_s[:], ALU.add)
            nc.scalar.dma_start(out=x1_out[:], in_=ar_col[:])
            sq2 = sp.tile([128, 16], f32, name="sq2")
            x2acc = sp.tile([128, 1], f32, name="x2acc")
            nc.scalar.activation(sq2[:], ar_col[:], AF.Square,
                                 accum_out=x2acc[:])
            ps_s2 = puno([1, 1], "ps_s2")
            nc.tensor.matmul(ps_s2[:], lhsT=ones_c[:], rhs=x2acc[:],
                             start=True, stop=True)
            s2p = sp.tile([1, 1], f32, name="s2p")
            nc.scalar.activation(s2p[:], ps_s2[:], AF.Ln, scale=1.0 / H,
                                 bias=eps1[:])
            s2 = sp.tile([1, 1], f32, name="s2")
            nc.scalar.activation(s2[:], s2p[:], AF.Exp, scale=-0.5)
            ps_s2b = puno([128, 1], "ps_s2b")
            nc.tensor.matmul(ps_s2b[:], lhsT=ones_r[:], rhs=s2[:],
                             start=True, stop=True)
            s2b = sp.tile([128, 1], f32, name="s2b")
            nc.scalar.copy(s2b[:], ps_s2b[:])
            h2 = sp.tile([128, 16], f32, name="h2")
            nc.vector.tensor_scalar(h2[:], ar_col[:], s2b[:], None, ALU.mult)
            h2c = sp.tile([128, 16], f32r, name="h2c")
            nc.vector.tensor_tensor(h2c[:], h2[:], w2f_s[:], ALU.mult)

            # ---- FFN gate/up matvecs ----
            ps_gu = pp.tile([1, 2 * IC], f32, tag="big4", bufs=1, name="ps_gu")
            for jt in range(4):
                wgt = wp.tile([128, 16, 256], f32r, tag="w", name="wgt")
                nc.sync.dma_start(out=wgt[:], in_=wg[:, :, jt * 256:(jt + 1) * 256])
                for c in range(16):
                    nc.tensor.matmul(ps_gu[0:1, jt * 256:(jt + 1) * 256],
                                     lhsT=h2c[:, c:c + 1], rhs=wgt[:, c, :],
                                     start=(c == 0), stop=(c == 15))
                wut = wp.tile([128, 16, 256], f32r, tag="w", name="wut")
                nc.sync.dma_start(out=wut[:], in_=wu[:, :, jt * 256:(jt + 1) * 256])
                for c in range(16):
                    nc.tensor.matmul(ps_gu[0:1, IC + jt * 256:IC + (jt + 1) * 256],
                                     lhsT=h2c[:, c:c + 1], rhs=wut[:, c, :],
                                     start=(c == 0), stop=(c == 15))
            # silu(g)*u in 256-chunks, alternating engines to pipeline
            s_row = sp.tile([1, IC], f32, name="s_row")
            for jt in range(4):
                gsl = s_row[0:1, jt * 256:(jt + 1) * 256]
                gch = ps_gu[0:1, jt * 256:(jt + 1) * 256]
                uch = ps_gu[0:1, IC + jt * 256:IC + (jt + 1) * 256]
                gs_row = sp.tile([1, 256], f32, tag="gs_row", name="gs_row",
                                 bufs=2)
                nc.scalar.activation(gs_row[:], gch, AF.Exp, scale=-1.0)
                nc.vector.tensor_scalar_add(gs_row[:], gs_row[:], 1.0)
                nc.vector.reciprocal(gs_row[:], gs_row[:])
                nc.vector.tensor_tensor(gs_row[:], gs_row[:], gch, ALU.mult)
                nc.vector.tensor_tensor(gsl, gs_row[:], uch, ALU.mult)
            ps_sT = puno([128, 8], "ps_sT")
            s_col = sp.tile([128, 8], f32r, name="s_col")
            for i in range(8):
                nc.tensor.transpose(ps_sT[:, i:i + 1],
                                    s_row[0:1, i * 128:(i + 1) * 128],
                                    ones11[:])
                if i % 2 == 0:
                    nc.scalar.copy(s_col[:, i:i + 1], ps_sT[:, i:i + 1])
                else:
                    nc.vector.tensor_copy(s_col[:, i:i + 1], ps_sT[:, i:i + 1])

            # ---- FFN down matvec -> partial [1,2048] ----
            ps_ffn = pp.tile([1, H], f32, tag="big4", bufs=1, name="ps_ffn")
            for jt in range(4):
                wdt = wp.tile([128, 8, 512], f32r, tag="w", name="wdt")
                nc.sync.dma_start(out=wdt[:], in_=wd[:, :, jt * 512:(jt + 1) * 512])
                for c in range(8):
                    nc.tensor.matmul(ps_ffn[0:1, jt * 512:(jt + 1) * 512],
                                     lhsT=s_col[:, c:c + 1],
                                     rhs=wdt[:, c, :],
                                     start=(c == 0), stop=(c == 7))
            ffn_row = sp.tile([1, H], f32, name="ffn_row")
            nc.scalar.copy(ffn_row[0:1, 0:512], ps_ffn[0:1, 0:512])
            nc.vector.tensor_copy(ffn_row[0:1, 512:1024], ps_ffn[0:1, 512:1024])
            nc.scalar.copy(ffn_row[0:1, 1024:1536], ps_ffn[0:1, 1024:1536])
            nc.vector.tensor_copy(ffn_row[0:1, 1536:2048],
                                  ps_ffn[0:1, 1536:2048])
            nc.scalar.dma_start(out=ffn_out[:], in_=ffn_row[:])

    nc.compile()
    return nc


_CACHE = {}


def _get_program():
    if "nc" not in _CACHE:
        _CACHE["nc"] = build_program()
    return _CACHE["nc"]


def _shard_inputs(x, conv_state, ssm_state, in_proj_w, out_proj_w, conv_weight,
                  A_log, dt_bias, norm_weight, rms1_w, rms2_w, gate_w, up_w,
                  down_w):
    """Pack + shard full inputs into per-core input maps."""
    f = np.float32
    in_maps = []
    ch_idx_all = []
    for c in range(NC):
        q_rows = np.arange(256 * c, 256 * (c + 1))
        k_rows = 2048 + q_rows
        v_rows = np.arange(4096 + 512 * c, 4096 + 512 * (c + 1))
        z_rows = np.arange(QKV + 512 * c, QKV + 512 * (c + 1))
        a_rows = np.arange(QKV + TV + 4 * c, QKV + TV + 4 * (c + 1))
        b_rows = a_rows + NVH
        rows1 = np.concatenate([q_rows, k_rows, v_rows, z_rows, a_rows, b_rows])
        ch_idx = np.concatenate([q_rows, k_rows, v_rows])  # conv channels
        ch_idx_all.append(ch_idx)
        vs = slice(512 * c, 512 * (c + 1))
        ics = slice(IC * c, IC * (c + 1))
        vh = slice(4 * c, 4 * (c + 1))

        cs = conv_state[ch_idx]          # [1024, 4]
        cw = conv_weight[ch_idx]         # [1024, 4]
        in_maps.append({
            "w1": _pack_T(in_proj_w[rows1]),
            "wo": _pack_T(out_proj_w[:, vs]),
            "wg": _pack_T(gate_w[ics]),
            "wu": _pack_T(up_w[ics]),
            "wd": _pack_T(down_w[:, ics]),
            "xc": _colmaj(x[0]).astype(f),
            "w1f": _colmaj(1.0 + rms1_w).astype(f),
            "w2f": _colmaj(1.0 + rms2_w).astype(f),
            "convp": _colmaj(np.sum(cs[:, 1:4] * cw[:, 0:3], axis=1)).astype(f),
            "cwlast": _colmaj(cw[:, 3]).astype(f),
            "ssm": np.concatenate([ssm_state[i] for i in range(4 * c, 4 * c + 4)],
                                  axis=1).astype(f),
            "scal": np.concatenate([A_log[vh], dt_bias[vh]])[None, :].astype(f),
            "qmask": np.array([[INV_SQRT_KD, INV_SQRT_KD, 1.0, 1.0]], f),
            "nwc": norm_weight[:, None].astype(f),
            "ident": np.eye(128, dtype=f),
        })
    return in_maps, ch_idx_all


def kernel(**inputs):
    inputs = {k: np.asarray(v) for k, v in inputs.items()}
    nc = _get_program()
    in_maps, ch_idx_all = _shard_inputs(**inputs)
    res = run_bass_kernel_spmd(nc, in_maps, core_ids=list(range(NC))).results

    # ---- unshard ----
    x_final = res[0]["x1_out"].T.reshape(1, H).astype(np.float64)
    for c in range(NC):
        x_final = x_final + res[c]["ffn_out"].astype(np.float64)
    x_final = x_final.astype(np.float32)

    conv_state = inputs["conv_state"]
    new_conv = np.empty((QKV, KC), np.float32)
    new_conv[:, 0:3] = conv_state[:, 1:4]
    for c in range(NC):
        new_conv[ch_idx_all[c], 3] = res[c]["qkv_out"].T.reshape(-1)

    new_ssm = np.empty((NVH, KD, VD), np.float32)
    for c in range(NC):
        blk = res[c]["ssm_out"].reshape(KD, N_VH, VD).transpose(1, 0, 2)
        new_ssm[4 * c:4 * (c + 1)] = blk

    return x_final, new_conv, new_ssm


# revision 11
# speedup vs baseline: 1.7001x; 1.1100x over previous
"""DeltaNet decode-step layer on 8 TRN2 NeuronCores (Bass/Tile).

Sharding (tensor-parallel over the 32 value heads + FFN intermediate):
  - core c owns value heads [4c, 4c+4) and key heads [2c, 2c+2)
  - in_proj rows / conv channels / z rows / a,b rows for those heads
  - out_proj columns for those heads (partial attn output, AllReduce'd)
  - FFN intermediate rows [1024c, 1024c+1024) (partial down output,
    summed on the host during unshard)

Weights are pre-transposed + packed on the host so every big matvec
streams the weight as the matmul *moving* operand in fp32r at
1 column/cycle:  pack[p, c, j] = W.T[c*128+p, j].

Elementwise chains run in partition-parallel "column" layouts
([128, nheads] tiles, one head per free column) — single-partition
[1, N] ops process only one lane/cycle and would dominate the
critical path otherwise.  PE transposes move rows<->columns.

A tiny dummy AllGather is issued first so the once-per-NEFF collective
entry barrier overlaps the in_proj weight streaming instead of the
real AllReduce.
"""
import numpy as np

import concourse.bacc as bacc
import concourse.mybir as mybir
import concourse.tile as tile
from concourse.bass_utils import run_bass_kernel_spmd

dt = mybir.dt
AF = mybir.ActivationFunctionType
ALU = mybir.AluOpType

H, I, QKV, TV = 2048, 8192, 8192, 4096
NVH, NKH, KD, VD, KC = 32, 16, 128, 128, 4
NC = 8
N_VH, N_KH = NVH // NC, NKH // NC          # 4 value heads, 2 key heads/core
CH = N_KH * KD * 2 + N_VH * VD             # 1024 conv channels/core
NZ = N_VH * VD                             # 512 z rows/core
R1 = CH + NZ + 2 * N_VH                    # 1544 in_proj rows/core
IC = I // NC                               # 1024 ffn rows/core
EPS = 1e-6
INV_SQRT_KD = float(KD) ** -0.5


def _pack_T(w_rows):
    """[J, H'] weight rows -> fp32 [128, H'/128, J] moving-operand pack."""
    t = np.ascontiguousarray(w_rows.T)                 # [H', J]
    hp, j = t.shape
    return np.ascontiguousarray(
        t.reshape(hp // 128, 128, j).transpose(1, 0, 2))


def _colmaj(v):
    """[N]-vector -> [128, N/128] column-major tile (elem i -> [i%128, i//128])."""
    return np.ascontiguousarray(v.reshape(-1, 128).T)


def build_program():
    nc = bacc.Bacc("TRN2", target_bir_lowering=False, debug=False,
                   num_devices=NC)
    f32, f32r = dt.float32, dt.float32r

    # ---- per-core DRAM inputs ----
    w1 = nc.dram_tensor("w1", [128, 16, R1], f32r, kind="ExternalInput")
    wo = nc.dram_tensor("wo", [128, 4, H], f32r, kind="ExternalInput")
    wg = nc.dram_tensor("wg", [128, 16, IC], f32r, kind="ExternalInput")
    wu = nc.dram_tensor("wu", [128, 16, IC], f32r, kind="ExternalInput")
    wd = nc.dram_tensor("wd", [128, 8, H], f32r, kind="ExternalInput")
    smalls = nc.dram_tensor("smalls", [128, 65], f32, kind="ExternalInput")
    vec16 = nc.dram_tensor("vec16", [1, 16], f32, kind="ExternalInput")
    ssm = nc.dram_tensor("ssm", [128, N_VH * VD], f32, kind="ExternalInput")
    ident = nc.dram_tensor("ident", [128, 128], f32, kind="ExternalInput")

    # ---- per-core DRAM outputs ----
    qkv_out = nc.dram_tensor("qkv_out", [128, 8], f32, kind="ExternalOutput")
    ssm_out = nc.dram_tensor("ssm_out", [128, N_VH * VD], f32,
                             kind="ExternalOutput")
    x1_out = nc.dram_tensor("x1_out", [128, 16], f32, kind="ExternalOutput")
    ffn_out = nc.dram_tensor("ffn_out", [1, H], f32, kind="ExternalOutput")
    dbg_out = nc.dram_tensor("dbg_out", [NC, 4], f32, kind="ExternalOutput")

    with tile.TileContext(nc, trace_sim=False) as tc:
        with tc.tile_pool(name="wp", bufs=8) as wp, \
             tc.tile_pool(name="sp", bufs=1) as sp, \
             tc.tile_pool(name="pp", bufs=1, space="PSUM") as pp, \
             tc.tile_pool(name="dp", bufs=1, space="DRAM") as dp:

            def puno(shape, name):
                return pp.tile(shape, f32, tag="uno", bufs=4, name=name)

            # ---- dummy collective: absorb the once-per-NEFF barrier ----
            dz = sp.tile([1, 4], f32, name="dz")
            nc.vector.memset(dz[:], 1.0)
            d_in = dp.tile([1, 4], f32, name="d_in")
            d_out = dp.tile([NC, 4], f32, name="d_out")
            nc.scalar.dma_start(out=d_in[:], in_=dz[:])
            nc.gpsimd.collective_compute(
                "AllGather", ALU.bypass, ins=[d_in.opt()], outs=[d_out.opt()],
                replica_groups=[list(range(NC))])
            dzo = sp.tile([NC, 4], f32, name="dzo")
            nc.gpsimd.dma_start(out=dzo[:], in_=d_out[:])
            nc.gpsimd.dma_start(out=dbg_out[:], in_=dzo[:])

            # ---- small input loads: 3 consolidated DMAs on the ACT ring ----
            sm = sp.tile([128, 65], f32, name="sm")
            nc.scalar.dma_start(out=sm[:], in_=smalls[:])
            v16 = sp.tile([1, 16], f32, name="v16")
            nc.scalar.dma_start(out=v16[:], in_=vec16[:])
            ssm_s = sp.tile([128, N_VH * VD], f32, name="ssm_s")
            nc.scalar.dma_start(out=ssm_s[:], in_=ssm[:])
            id_s = sp.tile([128, 128], f32, name="id_s")
            nc.scalar.dma_start(out=id_s[:], in_=ident[:])
            xc_s = sm[:, 0:16]
            w1f_s = sm[:, 16:32]
            w2f_s = sm[:, 32:48]
            convp_s = sm[:, 48:56]
            cwlast_s = sm[:, 56:64]
            nw_s = sm[:, 64:65]
            scal_s = v16[0:1, 0:8]
            qmask_s = v16[0:1, 8:12]

            ones11 = sp.tile([1, 1], f32, name="ones11")
            nc.vector.memset(ones11[:], 1.0)
            ones_r = sp.tile([1, 128], f32, name="ones_r")
            nc.vector.memset(ones_r[:], 1.0)
            ones_c = sp.tile([128, 1], f32, name="ones_c")
            nc.vector.memset(ones_c[:], 1.0)
            eps1 = sp.tile([1, 1], f32, name="eps1")
            nc.vector.memset(eps1[:], EPS)

            # ---- stage A: h = rms(x) * (1 + rms1_w), column layout ----
            sqA = sp.tile([128, 16], f32, name="sqA")
            xacc = sp.tile([128, 1], f32, name="xacc")
            nc.scalar.activation(sqA[:], xc_s, AF.Square, accum_out=xacc[:])
            ps_s1 = puno([1, 1], "ps_s1")
            nc.tensor.matmul(ps_s1[:], lhsT=ones_c[:], rhs=xacc[:],
                             start=True, stop=True)
            s1p = sp.tile([1, 1], f32, name="s1p")
            nc.scalar.activation(s1p[:], ps_s1[:], AF.Sqrt,
                                 scale=1.0 / H, bias=eps1[:])
            s1 = sp.tile([1, 1], f32, name="s1")
            nc.vector.reciprocal(s1[:], s1p[:])
            ps_s1b = puno([128, 1], "ps_s1b")
            nc.tensor.matmul(ps_s1b[:], lhsT=ones_r[:], rhs=s1[:],
                             start=True, stop=True)
            s1b = sp.tile([128, 1], f32, name="s1b")
            nc.scalar.copy(s1b[:], ps_s1b[:])
            h1 = sp.tile([128, 16], f32, name="h1")
            nc.vector.tensor_scalar(h1[:], xc_s, s1b[:], None, ALU.mult)
            h_r = sp.tile([128, 16], f32r, name="h_r")
            nc.vector.tensor_tensor(h_r[:], h1[:], w1f_s, ALU.mult)

            # ---- in_proj matvecs (fp32r, weight moving) ----
            # a/b rows first: decay/beta computed early off the critical path
            w1ab = wp.tile([128, 16, 2 * N_VH], f32r, tag="w", name="w1ab")
            nc.sync.dma_start(out=w1ab[:], in_=w1[:, :, CH + NZ:R1])
            ps_ab = puno([1, 2 * N_VH], "ps_ab")
            for c in range(16):
                nc.tensor.matmul(ps_ab[:], lhsT=h_r[:, c:c + 1],
                                 rhs=w1ab[:, c, :],
                                 start=(c == 0), stop=(c == 15))
            ab_row = sp.tile([1, 2 * N_VH], f32, name="ab_row")
            nc.scalar.copy(ab_row[:], ps_ab[:])

            # qk / v / z tiles; first two at 256-wide for an early PE start
            ps_qk = puno([1, 512], "ps_qk")
            ps_v = puno([1, 512], "ps_v")
            ps_z = puno([1, 512], "ps_z")
            jtiles = [(0, ps_qk, 0), (256, ps_qk, 256),
                      (512, ps_v, 0), (768, ps_v, 256),
                      (1024, ps_z, 0), (1280, ps_z, 256)]
            for j0, ps, off in jtiles:
                wt = wp.tile([128, 16, 256], f32r, tag="w", name="w1t")
                nc.sync.dma_start(out=wt[:], in_=w1[:, :, j0:j0 + 256])
                for c in range(16):
                    nc.tensor.matmul(ps[0:1, off:off + 256],
                                     lhsT=h_r[:, c:c + 1], rhs=wt[:, c, :],
                                     start=(c == 0), stop=(c == 15))

            qkv_row = sp.tile([1, CH], f32, name="qkv_row")
            nc.scalar.copy(qkv_row[0:1, 0:512], ps_qk[:])
            nc.vector.tensor_copy(qkv_row[0:1, 512:1024], ps_v[:])

            # ---- decay / beta (from a,b rows) ----
            beta = sp.tile([1, N_VH], f32, name="beta")
            nc.scalar.activation(beta[:], ab_row[0:1, N_VH:2 * N_VH], AF.Exp,
                                 scale=-1.0)
            nc.vector.tensor_scalar_add(beta[:], beta[:], 1.0)
            nc.vector.reciprocal(beta[:], beta[:])
            spin = sp.tile([1, N_VH], f32, name="spin")
            nc.vector.tensor_tensor(spin[:], ab_row[0:1, 0:N_VH],
                                    v16[0:1, 4:8], ALU.add)
            sp_t = sp.tile([1, N_VH], f32, name="sp_t")
            nc.scalar.activation(sp_t[:], spin[:], AF.Exp)
            nc.vector.tensor_scalar_add(sp_t[:], sp_t[:], 1.0)
            nc.scalar.activation(sp_t[:], sp_t[:], AF.Ln)
            eA = sp.tile([1, N_VH], f32, name="eA")
            nc.scalar.activation(eA[:], v16[0:1, 0:4], AF.Exp)
            gabs = sp.tile([1, N_VH], f32, name="gabs")
            nc.vector.tensor_tensor(gabs[:], eA[:], sp_t[:], ALU.mult)
            decay = sp.tile([1, N_VH], f32, name="decay")
            nc.scalar.activation(decay[:], gabs[:], AF.Exp, scale=-1.0)
            ps_db = puno([128, N_VH], "ps_db")
            nc.tensor.matmul(ps_db[:], lhsT=ones_r[:], rhs=decay[:],
                             start=True, stop=True)
            decay_bc = sp.tile([128, N_VH], f32, name="decay_bc")
            nc.scalar.copy(decay_bc[:], ps_db[:])
            decayed = sp.tile([128, N_VH * VD], f32, name="decayed")
            for vh in range(N_VH):
                nc.vector.tensor_scalar(
                    decayed[:, vh * VD:(vh + 1) * VD],
                    ssm_s[:, vh * VD:(vh + 1) * VD],
                    decay_bc[:, vh:vh + 1], None, ALU.mult)

            # ---- mixed_qkv row -> columns (one head per column) ----
            ps_qc = puno([128, 8], "ps_qc")
            for i in range(8):
                nc.tensor.transpose(ps_qc[:, i:i + 1],
                                    qkv_row[0:1, i * 128:(i + 1) * 128],
                                    ones11[:])
            qkv_col = sp.tile([128, 8], f32, name="qkv_col")
            nc.vector.tensor_copy(qkv_col[:], ps_qc[:])
            nc.scalar.dma_start(out=qkv_out[:], in_=qkv_col[:])

            # ---- conv step + silu, column space ----
            convs = sp.tile([128, 8], f32, name="convs")
            nc.vector.tensor_tensor(convs[:], qkv_col[:], cwlast_s, ALU.mult)
            nc.vector.tensor_tensor(convs[:], convs[:], convp_s, ALU.add)
            conv_c = sp.tile([128, 8], f32, name="conv_c")
            nc.scalar.activation(conv_c[:], convs[:], AF.Exp, scale=-1.0)
            nc.vector.tensor_scalar_add(conv_c[:], conv_c[:], 1.0)
            nc.vector.reciprocal(conv_c[:], conv_c[:])
            nc.vector.tensor_tensor(conv_c[:], conv_c[:], convs[:], ALU.mult)

            # ---- silu(z) gate (col) ----
            z_row = sp.tile([1, 512], f32, name="z_row")
            nc.scalar.copy(z_row[:], ps_z[:])
            ps_zc = puno([128, 4], "ps_zc")
            for i in range(4):
                nc.tensor.transpose(ps_zc[:, i:i + 1],
                                    z_row[0:1, i * 128:(i + 1) * 128],
                                    ones11[:])
            z_col = sp.tile([128, 4], f32, name="z_col")
            nc.vector.tensor_copy(z_col[:], ps_zc[:])
            zs_col = sp.tile([128, 4], f32, name="zs_col")
            nc.scalar.activation(zs_col[:], z_col[:], AF.Exp, scale=-1.0)
            nc.vector.tensor_scalar_add(zs_col[:], zs_col[:], 1.0)
            nc.vector.reciprocal(zs_col[:], zs_col[:])
            nc.vector.tensor_tensor(zs_col[:], zs_col[:], z_col[:], ALU.mult)

            # ---- l2-normalize q,k columns (cols 0,1=q; 2,3=k) ----
            sq4 = sp.tile([128, 4], f32, name="sq4")
            nc.vector.tensor_tensor(sq4[:], conv_c[:, 0:4], conv_c[:, 0:4],
                                    ALU.mult)
            ps_n4 = puno([1, 4], "ps_n4")
            nc.tensor.matmul(ps_n4[:], lhsT=ones_c[:], rhs=sq4[:],
                             start=True, stop=True)
            ss4 = sp.tile([1, 4], f32, name="ss4")
            nc.scalar.activation(ss4[:], ps_n4[:], AF.Sqrt)
            nc.vector.tensor_scalar_max(ss4[:], ss4[:], 1e-12)
            nc.vector.reciprocal(ss4[:], ss4[:])
            nc.vector.tensor_tensor(ss4[:], ss4[:], qmask_s, ALU.mult)
            ps_n4b = puno([128, 4], "ps_n4b")
            nc.tensor.matmul(ps_n4b[:], lhsT=ones_r[:], rhs=ss4[:],
                             start=True, stop=True)
            qkn = sp.tile([128, 4], f32, name="qkn")
            nc.vector.tensor_tensor(qkn[:], conv_c[:, 0:4], ps_n4b[:], ALU.mult)

            # ---- delta rule, column space ----
            # Sk_col[:, vh] = decayed_blk(vh).T @ k_col(vh//2)
            ps_skc = puno([128, 4], "ps_skc")
            for vh in range(N_VH):
                nc.tensor.matmul(ps_skc[:, vh:vh + 1],
                                 lhsT=decayed[:, vh * VD:(vh + 1) * VD],
                                 rhs=qkn[:, 2 + vh // 2:3 + vh // 2],
                                 start=True, stop=True)
            ps_bb = puno([128, 4], "ps_bb")
            nc.tensor.matmul(ps_bb[:], lhsT=ones_r[:], rhs=beta[:],
                             start=True, stop=True)
            delta_c = sp.tile([128, 4], f32, name="delta_c")
            nc.vector.tensor_tensor(delta_c[:], conv_c[:, 4:8], ps_skc[:],
                                    ALU.subtract)
            nc.vector.tensor_tensor(delta_c[:], delta_c[:], ps_bb[:], ALU.mult)
            # rows needed for the rank-1 outer products
            ps_dr = puno([1, 512], "ps_dr")
            for i in range(4):
                nc.tensor.transpose(ps_dr[0:1, i * 128:(i + 1) * 128],
                                    delta_c[:, i:i + 1], id_s)
            delta_r = sp.tile([1, 512], f32, name="delta_r")
            nc.scalar.copy(delta_r[:], ps_dr[:])
            ps_kr = puno([1, 256], "ps_kr")
            for i in range(2):
                nc.tensor.transpose(ps_kr[0:1, i * 128:(i + 1) * 128],
                                    qkn[:, 2 + i:3 + i], id_s)
            k_row = sp.tile([1, 256], f32, name="k_row")
            nc.vector.tensor_copy(k_row[:], ps_kr[:])

            ps_ssm = puno([128, 512], "ps_ssm")
            for vh in range(N_VH):
                nc.tensor.matmul(ps_ssm[:, vh * 128:(vh + 1) * 128],
                                 lhsT=k_row[0:1, (vh // 2) * 128:
                                            (vh // 2 + 1) * 128],
                                 rhs=delta_r[0:1, vh * 128:(vh + 1) * 128],
                                 start=True, stop=True)
            new_ssm = sp.tile([128, 512], f32, name="new_ssm")
            nc.vector.tensor_tensor(new_ssm[:], ps_ssm[:], decayed[:], ALU.add)
            nc.scalar.dma_start(out=ssm_out[:], in_=new_ssm[:])

            # y_col[:, vh] = new_ssm_blk(vh).T @ q_col(vh//2)
            ps_yc = puno([128, 4], "ps_yc")
            for vh in range(N_VH):
                nc.tensor.matmul(ps_yc[:, vh:vh + 1],
                                 lhsT=new_ssm[:, vh * 128:(vh + 1) * 128],
                                 rhs=qkn[:, vh // 2:vh // 2 + 1],
                                 start=True, stop=True)
            y_col = sp.tile([128, 4], f32, name="y_col")
            nc.scalar.copy(y_col[:], ps_yc[:])

            # ---- gated rmsnorm (col) ----
            ysq = sp.tile([128, 4], f32, name="ysq")
            nc.vector.tensor_tensor(ysq[:], y_col[:], y_col[:], ALU.mult)
            ps_y4 = puno([1, 4], "ps_y4")
            nc.tensor.matmul(ps_y4[:], lhsT=ones_c[:], rhs=ysq[:],
                             start=True, stop=True)
            ys4 = sp.tile([1, 4], f32, name="ys4")
            nc.scalar.activation(ys4[:], ps_y4[:], AF.Sqrt, scale=1.0 / VD,
                                 bias=eps1[:])
            nc.vector.reciprocal(ys4[:], ys4[:])
            ps_y4b = puno([128, 4], "ps_y4b")
            nc.tensor.matmul(ps_y4b[:], lhsT=ones_r[:], rhs=ys4[:],
                             start=True, stop=True)
            nc.vector.tensor_tensor(y_col[:], y_col[:], ps_y4b[:], ALU.mult)
            nc.vector.tensor_scalar(y_col[:], y_col[:], nw_s, None, ALU.mult)

            yo_col = sp.tile([128, 4], f32r, name="yo_col")
            nc.vector.tensor_tensor(yo_col[:], y_col[:], zs_col[:], ALU.mult)

            # ---- out_proj partial -> [1,2048] row ----
            ps_attn = pp.tile([1, H], f32, tag="big4", bufs=1, name="ps_attn")
            for half in range(2):
                wot = wp.tile([128, 4, 1024], f32r, tag="w", name="wot")
                nc.sync.dma_start(out=wot[:],
                                  in_=wo[:, :, half * 1024:(half + 1) * 1024])
                for sub in range(2):
                    off = half * 1024 + sub * 512
                    for c in range(4):
                        nc.tensor.matmul(
                            ps_attn[0:1, off:off + 512],
                            lhsT=yo_col[:, c:c + 1],
                            rhs=wot[:, c, sub * 512:(sub + 1) * 512],
                            start=(c == 0), stop=(c == 3))
            attn_row = sp.tile([1, H], f32, name="attn_row")
            nc.scalar.copy(attn_row[0:1, 0:512], ps_attn[0:1, 0:512])
            nc.vector.tensor_copy(attn_row[0:1, 512:1024], ps_attn[0:1, 512:1024])
            nc.scalar.copy(attn_row[0:1, 1024:1536], ps_attn[0:1, 1024:1536])
            nc.vector.tensor_copy(attn_row[0:1, 1536:2048],
                                  ps_attn[0:1, 1536:2048])
            ps_ac = puno([128, 16], "ps_ac")
            for i in range(16):
                nc.tensor.transpose(ps_ac[:, i:i + 1],
                                    attn_row[0:1, i * 128:(i + 1) * 128],
                                    ones11[:])
            ac_col = sp.tile([128, 16], f32, name="ac_col")
            nc.vector.tensor_copy(ac_col[:], ps_ac[:])

            # ---- AllReduce attn partials (column layout) ----
            ar_in = dp.tile([128, 16], f32, name="ar_in")
            ar_out = dp.tile([128, 16], f32, name="ar_out")
            nc.scalar.dma_start(out=ar_in[:], in_=ac_col[:])
            nc.gpsimd.collective_compute(
                "AllReduce", ALU.add, ins=[ar_in.opt()], outs=[ar_out.opt()],
                replica_groups=[list(range(NC))])
            ar_col = sp.tile([128, 16], f32, name="ar_col")
            nc.scalar.dma_start(out=ar_col[:], in_=ar_out[:])

            # ---- residual + rms2 (col) ----
            nc.vector.tensor_tensor(ar_col[:], ar_col[:], xc_s, ALU.add)
            nc.scalar.dma_start(out=x1_out[:], in_=ar_col[:])
            sq2 = sp.tile([128, 16], f32, name="sq2")
            x2acc = sp.tile([128, 1], f32, name="x2acc")
            nc.scalar.activation(sq2[:], ar_col[:], AF.Square,
                                 accum_out=x2acc[:])
            ps_s2 = puno([1, 1], "ps_s2")
            nc.tensor.matmul(ps_s2[:], lhsT=ones_c[:], rhs=x2acc[:],
                             start=True, stop=True)
            s2p = sp.tile([1, 1], f32, name="s2p")
            nc.scalar.activation(s2p[:], ps_s2[:], AF.Sqrt, scale=1.0 / H,
                                 bias=eps1[:])
            s2 = sp.tile([1, 1], f32, name="s2")
            nc.vector.reciprocal(s2[:], s2p[:])
            ps_s2b = puno([128, 1], "ps_s2b")
            nc.tensor.matmul(ps_s2b[:], lhsT=ones_r[:], rhs=s2[:],
                             start=True, stop=True)
            s2b = sp.tile([128, 1], f32, name="s2b")
            nc.scalar.copy(s2b[:], ps_s2b[:])
            h2 = sp.tile([128, 16], f32, name="h2")
            nc.vector.tensor_scalar(h2[:], ar_col[:], s2b[:], None, ALU.mult)
            h2c = sp.tile([128, 16], f32r, name="h2c")
            nc.vector.tensor_tensor(h2c[:], h2[:], w2f_s, ALU.mult)

            # ---- FFN gate/up matvecs ----
            ps_gu = pp.tile([1, 2 * IC], f32, tag="big4", bufs=1, name="ps_gu")
            for jt in range(4):
                wgt = wp.tile([128, 16, 256], f32r, tag="w", name="wgt")
                nc.sync.dma_start(out=wgt[:], in_=wg[:, :, jt * 256:(jt + 1) * 256])
                for c in range(16):
                    nc.tensor.matmul(ps_gu[0:1, jt * 256:(jt + 1) * 256],
                                     lhsT=h2c[:, c:c + 1], rhs=wgt[:, c, :],
                                     start=(c == 0), stop=(c == 15))
                wut = wp.tile([128, 16, 256], f32r, tag="w", name="wut")
                nc.sync.dma_start(out=wut[:], in_=wu[:, :, jt * 256:(jt + 1) * 256])
                for c in range(16):
                    nc.tensor.matmul(ps_gu[0:1, IC + jt * 256:IC + (jt + 1) * 256],
                                     lhsT=h2c[:, c:c + 1], rhs=wut[:, c, :],
                                     start=(c == 0), stop=(c == 15))
            # silu(g)*u in 256-chunks, alternating engines to pipeline
            s_row = sp.tile([1, IC], f32, name="s_row")
            for jt in range(4):
                gsl = s_row[0:1, jt * 256:(jt + 1) * 256]
                gch = ps_gu[0:1, jt * 256:(jt + 1) * 256]
                uch = ps_gu[0:1, IC + jt * 256:IC + (jt + 1) * 256]
                gs_row = sp.tile([1, 256], f32, tag="gs_row", name="gs_row",
                                 bufs=2)
                nc.scalar.activation(gs_row[:], gch, AF.Exp, scale=-1.0)
                nc.vector.tensor_scalar_add(gs_row[:], gs_row[:], 1.0)
                nc.vector.reciprocal(gs_row[:], gs_row[:])
                nc.vector.tensor_tensor(gs_row[:], gs_row[:], gch, ALU.mult)
                nc.vector.tensor_tensor(gsl, gs_row[:], uch, ALU.mult)
            ps_sT = puno([128, 8], "ps_sT")
            s_col = sp.tile([128, 8], f32r, name="s_col")
            for i in range(8):
                nc.tensor.transpose(ps_sT[:, i:i + 1],
                                    s_row[0:1, i * 128:(i + 1) * 128],
                                    ones11[:])
                if i % 2 == 0:
                    nc.scalar.copy(s_col[:, i:i + 1], ps_sT[:, i:i + 1])
                else:
                    nc.vector.tensor_copy(s_col[:, i:i + 1], ps_sT[:, i:i + 1])

            # ---- FFN down matvec -> partial [1,2048] ----
            ps_ffn = pp.tile([1, H], f32, tag="big4", bufs=1, name="ps_ffn")
            for jt in range(4):
                wdt = wp.tile([128, 8, 512], f32r, tag="w", name="wdt")
                nc.sync.dma_start(out=wdt[:], in_=wd[:, :, jt * 512:(jt + 1) * 512])
                for c in range(8):
                    nc.tensor.matmul(ps_ffn[0:1, jt * 512:(jt + 1) * 512],
                                     lhsT=s_col[:, c:c + 1],
                                     rhs=wdt[:, c, :],
                                     start=(c == 0), stop=(c == 7))
            ffn_row = sp.tile([1, H], f32, name="ffn_row")
            nc.scalar.copy(ffn_row[0:1, 0:512], ps_ffn[0:1, 0:512])
            nc.vector.tensor_copy(ffn_row[0:1, 512:1024], ps_ffn[0:1, 512:1024])
            nc.scalar.copy(ffn_row[0:1, 1024:1536], ps_ffn[0:1, 1024:1536])
            nc.vector.tensor_copy(ffn_row[0:1, 1536:2048],
                                  ps_ffn[0:1, 1536:2048])
            nc.scalar.dma_start(out=ffn_out[:], in_=ffn_row[:])

    nc.compile()
    return nc


_CACHE = {}


def _get_program():
    if "nc" not in _CACHE:
        _CACHE["nc"] = build_program()
    return _CACHE["nc"]


def _shard_inputs(x, conv_state, ssm_state, in_proj_w, out_proj_w, conv_weight,
                  A_log, dt_bias, norm_weight, rms1_w, rms2_w, gate_w, up_w,
                  down_w):
    """Pack + shard full inputs into per-core input maps."""
    f = np.float32
    in_maps = []
    ch_idx_all = []
    for c in range(NC):
        q_rows = np.arange(256 * c, 256 * (c + 1))
        k_rows = 2048 + q_rows
        v_rows = np.arange(4096 + 512 * c, 4096 + 512 * (c + 1))
        z_rows = np.arange(QKV + 512 * c, QKV + 512 * (c + 1))
        a_rows = np.arange(QKV + TV + 4 * c, QKV + TV + 4 * (c + 1))
        b_rows = a_rows + NVH
        rows1 = np.concatenate([q_rows, k_rows, v_rows, z_rows, a_rows, b_rows])
        ch_idx = np.concatenate([q_rows, k_rows, v_rows])  # conv channels
        ch_idx_all.append(ch_idx)
        vs = slice(512 * c, 512 * (c + 1))
        ics = slice(IC * c, IC * (c + 1))
        vh = slice(4 * c, 4 * (c + 1))

        cs = conv_state[ch_idx]          # [1024, 4]
        cw = conv_weight[ch_idx]         # [1024, 4]
        in_maps.append({
            "w1": _pack_T(in_proj_w[rows1]),
            "wo": _pack_T(out_proj_w[:, vs]),
            "wg": _pack_T(gate_w[ics]),
            "wu": _pack_T(up_w[ics]),
            "wd": _pack_T(down_w[:, ics]),
            "smalls": np.concatenate([
                _colmaj(x[0]), _colmaj(1.0 + rms1_w), _colmaj(1.0 + rms2_w),
                _colmaj(np.sum(cs[:, 1:4] * cw[:, 0:3], axis=1)),
                _colmaj(cw[:, 3]), norm_weight[:, None]],
                axis=1).astype(f),
            "vec16": np.concatenate([
                A_log[vh], dt_bias[vh],
                [INV_SQRT_KD, INV_SQRT_KD, 1.0, 1.0],
                np.zeros(4)])[None, :].astype(f),
            "ssm": np.concatenate([ssm_state[i] for i in range(4 * c, 4 * c + 4)],
                                  axis=1).astype(f),
            "ident": np.eye(128, dtype=f),
        })
    return in_maps, ch_idx_all


def kernel(**inputs):
    inputs = {k: np.asarray(v) for k, v in inputs.items()}
    nc = _get_program()
    in_maps, ch_idx_all = _shard_inputs(**inputs)
    res = run_bass_kernel_spmd(nc, in_maps, core_ids=list(range(NC))).results

    # ---- unshard ----
    x_final = res[0]["x1_out"].T.reshape(1, H).astype(np.float64)
    for c in range(NC):
        x_final = x_final + res[c]["ffn_out"].astype(np.float64)
    x_final = x_final.astype(np.float32)

    conv_state = inputs["conv_state"]
    new_conv = np.empty((QKV, KC), np.float32)
    new_conv[:, 0:3] = conv_state[:, 1:4]
    for c in range(NC):
        new_conv[ch_idx_all[c], 3] = res[c]["qkv_out"].T.reshape(-1)

    new_ssm = np.empty((NVH, KD, VD), np.float32)
    for c in range(NC):
        blk = res[c]["ssm_out"].reshape(KD, N_VH, VD).transpose(1, 0, 2)
        new_ssm[4 * c:4 * (c + 1)] = blk

    return x_final, new_conv, new_ssm
